# revision 9
# baseline (speedup 1.0000x reference)
"""Trainium2 Bass kernel for nn_EpisodicMemory (scatter_memory).

Sharding: pure batch data-parallelism. 8 cores, 32 streams -> 4 streams/core.
Each core runs the full per-stream pipeline:
  q projections (PE) -> masked cosine scores over M=32768 slots (DVE
  tensor_tensor_reduce, em_K consumed in natural [slot, d] layout, em_S mask
  folded in as the reduce init scalar) -> per-partition top-8 (DVE Max8) ->
  batched fold to top-32 -> chained indirect DMA gathers (index table, em_V
  rows) -> cross-attention + softmax + FFN epilogue (PE/ACT, tiny).

`stage` (debug): 1 = scoring only (dump scores), 2 = + selection/gather
(dump V_top), 99 = full.
"""

import os
import sys

import numpy as np

sys.path.insert(0, "/opt/trn_rl_repo")

import concourse.bass as bass  # noqa: F401
import concourse.tile as tile
from concourse import bacc, mybir
from concourse.bass import IndirectOffsetOnAxis
from concourse.masks import make_identity

F32 = mybir.dt.float32
I32 = mybir.dt.int32
U32 = mybir.dt.uint32
OP = mybir.AluOpType
AF = mybir.ActivationFunctionType

NCORES = 8
BS, D, DE, KRET = 32, 1024, 128, 32
S = BS // NCORES  # streams per core = 4
NEG = -3.0e30  # stand-in for -inf (safe for exp/compare, no NaNs)


def register_dot_prefix():
    """Custom DVE op: out = running prefix-sum of Src0*Src1 along the free
    stream. With a stride-0 innermost out AP, the surviving write per page
    is the prefix total at that page's end -> segmented dot products in one
    instruction per chunk (vs one scalar_tensor_tensor + accum-read per
    128-slot column)."""
    from concourse.dve_ops import (
        CUSTOM_DVE_SPECS,
        OPS,
        _CUSTOM_DVE_ROW_BASE,
        _SUB_OPCODE_FOR_NAME,
        DveOp,
    )
    from concourse.dve_spec import AluOp, Spec, Src0, Src1, lower, scan
    from concourse.dve_uop import DveOpSpec

    name = "DOT_PREFIX_ANT"
    if name in _SUB_OPCODE_FOR_NAME:
        return next(op for op in OPS if op.name == name)

    def _ref(in0, in1, s0, s1, imm2):
        p = in0.shape[0]
        a = np.asarray(in0, np.float32).reshape(p, -1)
        b = np.asarray(in1, np.float32).reshape(p, -1)
        return np.cumsum(a * b, axis=-1, dtype=np.float32).reshape(in0.shape)

    spec = Spec(body=scan(AluOp.ADD, Src0 * Src1), reference=_ref)
    row = _CUSTOM_DVE_ROW_BASE + len(OPS)
    sha = {}
    for ver in ("v3", "v4"):
        tmp = DveOpSpec(name=name, opcode=row, uops=lower(spec, ver=ver), rd1_en=True)
        sha[ver] = tmp.sha(ver)
    op = DveOp(name, spec, subdim=False, uops_sha=sha)
    OPS.append(op)
    CUSTOM_DVE_SPECS[name] = spec
    _SUB_OPCODE_FOR_NAME[name] = row
    return op


def build_nc(M=32768, debug=False, act_fn=None, stage=99, reps=1, serial_reps=False):
    """Build the per-core Bass kernel. M = slots per stream (param for sim)."""
    if act_fn is None:
        act_fn = AF.Gelu
    CH = min(4096, M)         # slots per DMA chunk (4096 slots = 2 MB)
    NCHUNK = M // CH
    JPB = CH // 128           # rows per partition per chunk (32)
    NCOL = M // 128           # score columns (256)
    NCAND = 1024              # per-stream candidates (128 partitions x 8)

    dot_op = register_dot_prefix()
    nc = bacc.Bacc("TRN2", target_bir_lowering=False, debug=debug)

    # ---- DRAM I/O (per-core shard) ----
    d_x = nc.dram_tensor("x", [S, D], F32, kind="ExternalInput").ap()
    d_y = nc.dram_tensor("y_wm", [S, D], F32, kind="ExternalInput").ap()
    d_K = nc.dram_tensor("em_K", [S * M, DE], F32, kind="ExternalInput").ap()
    d_V = nc.dram_tensor("em_V", [S * M, DE], F32, kind="ExternalInput").ap()
    d_S = nc.dram_tensor("em_S", [S, M], F32, kind="ExternalInput").ap()
    d_wqe = nc.dram_tensor("Wq_em_w", [2 * D, DE], F32, kind="ExternalInput").ap()
    d_bqe = nc.dram_tensor("Wq_em_b", [DE], F32, kind="ExternalInput").ap()
    d_wqc = nc.dram_tensor("Wq_cross_w", [D, DE], F32, kind="ExternalInput").ap()
    d_bqc = nc.dram_tensor("Wq_cross_b", [DE], F32, kind="ExternalInput").ap()
    d_wo = nc.dram_tensor("Wo_w", [DE, D], F32, kind="ExternalInput").ap()
    d_bo = nc.dram_tensor("Wo_b", [D], F32, kind="ExternalInput").ap()
    d_lng = nc.dram_tensor("ln_g", [DE], F32, kind="ExternalInput").ap()
    d_lnb = nc.dram_tensor("ln_b", [DE], F32, kind="ExternalInput").ap()
    d_w1 = nc.dram_tensor("ffn1_w", [DE, 4 * DE], F32, kind="ExternalInput").ap()
    d_b1 = nc.dram_tensor("ffn1_b", [4 * DE], F32, kind="ExternalInput").ap()
    d_w2 = nc.dram_tensor("ffn2_w", [4 * DE, DE], F32, kind="ExternalInput").ap()
    d_b2 = nc.dram_tensor("ffn2_b", [DE], F32, kind="ExternalInput").ap()
    d_out = nc.dram_tensor("out", [S, D], F32, kind="ExternalOutput").ap()
    d_ident = nc.dram_tensor("cst_ident", [128, 128], F32, kind="ExternalInput").ap()
    d_iotaj = nc.dram_tensor("cst_iota_jpb", [128, 1], U32, kind="ExternalInput").ap()
    d_iotas = nc.dram_tensor("cst_iota_s", [S, 1], U32, kind="ExternalInput").ap()
    # index table for the chained gather (slot row ids as uint32)
    d_gtab = nc.dram_tensor("gtab", [S * NCAND, 1], U32).ap()

    with tile.TileContext(nc) as tc:
        with (
            tc.tile_pool(name="kpool", bufs=7) as kpool,
            tc.tile_pool(name="wpool", bufs=1) as wpool,
            tc.tile_pool(name="spool", bufs=1) as spool,
            tc.tile_pool(name="scr", bufs=2) as scr,
            tc.tile_pool(name="small", bufs=4) as small,
            tc.tile_pool(name="pp", bufs=3, space="PSUM") as pp,
            tc.tile_pool(name="pacc", bufs=2, space="PSUM") as pacc,
            tc.tile_pool(name="pq", bufs=2, space="PSUM") as pq,
        ):
            # ---- constants / weights in SBUF ----
            ident = wpool.tile([128, 128], F32, name="ident")
            nc.sync.dma_start(ident, d_ident)
            ones_row = wpool.tile([1, 128], F32, name="ones_row")
            nc.vector.memset(ones_row, 1.0)
            ones_col = wpool.tile([128, 1], F32, name="ones_col")
            nc.vector.memset(ones_col, 1.0)
            iota32 = wpool.tile([128, 1], U32, name="iota32")  # p * JPB
            nc.sync.dma_start(iota32, d_iotaj)
            iotaS = wpool.tile([S, 1], U32, name="iotaS")  # s * NCAND
            nc.sync.dma_start(iotaS, d_iotas)
            eps12 = wpool.tile([128, 1], F32, name="eps12")
            nc.vector.memset(eps12, 1e-12)
            eps5 = wpool.tile([128, 1], F32, name="eps5")
            nc.vector.memset(eps5, 1e-5)

            # Wq_em rows 2048 -> [128, 16*128]; Wq_cross rows 1024 -> [128, 8*128]
            wqe = wpool.tile([128, 16 * DE], F32, name="wqe")
            nc.sync.dma_start(wqe, d_wqe.rearrange("(j p) e -> p j e", p=128))
            wqc = wpool.tile([128, 8 * DE], F32, name="wqc")
            nc.sync.dma_start(wqc, d_wqc.rearrange("(j p) e -> p j e", p=128))
            w1 = wpool.tile([128, 512], F32, name="w1")
            nc.sync.dma_start(w1, d_w1)
            w2 = wpool.tile([128, 4 * DE], F32, name="w2")
            nc.sync.dma_start(w2, d_w2.rearrange("(k p) e -> p k e", p=128))
            wo = wpool.tile([128, D], F32, name="wo")
            nc.sync.dma_start(wo, d_wo)
            bqe_c = wpool.tile([128, 1], F32, name="bqe_c")
            nc.sync.dma_start(bqe_c, d_bqe[:, None])
            bqc_c = wpool.tile([128, 1], F32, name="bqc_c")
            nc.sync.dma_start(bqc_c, d_bqc[:, None])
            lng_c = wpool.tile([128, 1], F32, name="lng_c")
            nc.sync.dma_start(lng_c, d_lng[:, None])
            lnb_c = wpool.tile([128, 1], F32, name="lnb_c")
            nc.sync.dma_start(lnb_c, d_lnb[:, None])
            b1_c = wpool.tile([128, 4], F32, name="b1_c")
            nc.sync.dma_start(b1_c, d_b1.rearrange("(k p) -> p k", p=128))
            b2_c = wpool.tile([128, 1], F32, name="b2_c")
            nc.sync.dma_start(b2_c, d_b2[:, None])
            bo4 = wpool.tile([S, D], F32, name="bo4")
            for _s in range(S):
                nc.sync.dma_start(bo4[_s:_s + 1, :], d_bo[None, :])

            def bcast_col(val11, n=128):
                """[1,1] sbuf -> [n,1] sbuf via PE outer product."""
                ps = pp.tile([128, 1], F32, space="PSUM", tag="tr")
                nc.tensor.matmul(ps[:n, :], lhsT=ones_row[:, :n], rhs=val11,
                                 start=True, stop=True)
                sb = small.tile([n, 1], F32, tag="bc_sb")
                nc.vector.tensor_copy(sb, ps[:n, :])
                return sb

            def transpose(src, pdim, fdim):
                """[pdim, fdim] -> psum [fdim, pdim]; returns psum AP."""
                ps = pp.tile([128, 128], F32, space="PSUM", tag="tr")
                nc.tensor.transpose(ps[:fdim, :pdim], src, ident[:pdim, :pdim])
                return ps[:fdim, :pdim]

            def rsqrt11(val11, eps_ap, tag):
                """[1,1] -> 1/sqrt(val + eps) via exp(-0.5 * ln(val + eps))."""
                t = small.tile([1, 1], F32, tag=tag + "_ln")
                nc.scalar.activation(t, val11, AF.Ln, bias=eps_ap)
                t2 = small.tile([1, 1], F32, tag=tag + "_sc")
                nc.vector.tensor_scalar(t2, t, -0.5, None, op0=OP.mult)
                r = small.tile([1, 1], F32, tag=tag + "_ex")
                nc.scalar.activation(r, t2, AF.Exp)
                return r

            for rep_ in range(reps):
                if serial_reps and rep_ > 0:
                    fence = scr.tile([S, D], F32, tag="fence")
                    nc.sync.dma_start(fence, d_out)
                    fs = small.tile([S, 1], F32, tag="fs")
                    nc.vector.reduce_max(fs, fence, axis=mybir.AxisListType.X)
                # ---- phase 0: per-stream queries ----
                q_rep, qc_rep = [], []
                for s in range(S):
                    xn = scr.tile([8, 128], F32, tag="xn")
                    nc.sync.dma_start(xn, d_x[s].rearrange("(j d) -> j d", j=8))
                    yn = scr.tile([8, 128], F32, tag="yn")
                    nc.sync.dma_start(yn, d_y[s].rearrange("(j d) -> j d", j=8))
                    xT_ps = transpose(xn, 8, 128)
                    xT = scr.tile([128, 8], F32, tag="xT")
                    nc.vector.tensor_copy(xT, xT_ps)
                    yT_ps = transpose(yn, 8, 128)
                    yT = scr.tile([128, 8], F32, tag="yT")
                    nc.vector.tensor_copy(yT, yT_ps)

                    # q_em = normalize(concat(x,y) @ Wq_em + b)
                    ps_q = pacc.tile([128, 1], F32, space="PSUM", tag="acc")
                    for j in range(16):
                        rhs = xT[:, j:j + 1] if j < 8 else yT[:, j - 8:j - 7]
                        nc.tensor.matmul(ps_q, lhsT=wqe[:, j * DE:(j + 1) * DE], rhs=rhs,
                                         start=(j == 0), stop=(j == 15))
                    q_sb = small.tile([128, 1], F32, tag="q_sb")
                    nc.vector.tensor_add(q_sb, ps_q, bqe_c)
                    ps_n = pq.tile([1, 1], F32, space="PSUM", tag="row")
                    nc.tensor.matmul(ps_n, lhsT=q_sb, rhs=q_sb, start=True, stop=True)
                    n2 = small.tile([1, 1], F32, tag="n2")
                    nc.vector.tensor_copy(n2, ps_n)
                    inv = rsqrt11(n2, eps12[:1, :], tag="qinv")
                    inv_b = bcast_col(inv)
                    qhat = small.tile([128, 1], F32, tag="qhat")
                    nc.vector.tensor_mul(qhat, q_sb, inv_b)
                    qrow_ps = transpose(qhat, 128, 1)
                    qrow = small.tile([1, 128], F32, tag="qrow")
                    nc.vector.tensor_copy(qrow, qrow_ps)
                    ps_qr = pp.tile([128, 128], F32, space="PSUM", tag="tr")
                    nc.tensor.matmul(ps_qr, lhsT=ones_row, rhs=qrow, start=True, stop=True)
                    qr = spool.tile([128, 128], F32, name=f"q_rep{s}", tag=f"q_rep{s}")
                    nc.vector.tensor_copy(qr, ps_qr)
                    q_rep.append(qr)

                    # q_cross = x @ Wq_cross + b
                    ps_qc = pacc.tile([128, 1], F32, space="PSUM", tag="acc")
                    for j in range(8):
                        nc.tensor.matmul(ps_qc, lhsT=wqc[:, j * DE:(j + 1) * DE],
                                         rhs=xT[:, j:j + 1], start=(j == 0), stop=(j == 7))
                    qc_sb = small.tile([128, 1], F32, tag="qc_sb")
                    nc.vector.tensor_add(qc_sb, ps_qc, bqc_c)
                    qcrow_ps = transpose(qc_sb, 128, 1)
                    qcrow = small.tile([1, 128], F32, tag="qcrow")
                    nc.vector.tensor_copy(qcrow, qcrow_ps)
                    ps_qcr = pp.tile([128, 128], F32, space="PSUM", tag="tr")
                    nc.tensor.matmul(ps_qcr[:KRET, :], lhsT=ones_row[:, :KRET], rhs=qcrow,
                                     start=True, stop=True)
                    qcr = spool.tile([KRET, 128], F32, name=f"qc_rep{s}", tag=f"qc_rep{s}")
                    nc.vector.tensor_copy(qcr, ps_qcr[:KRET, :])
                    qc_rep.append(qcr)

                if stage == 0:
                    for s in range(S):
                        nc.sync.dma_start(
                            d_out[s:s + 1, :].rearrange("one (p r) -> p one r", p=128),
                            q_rep[s][:, :8])

                # ---- masks (em_S <= 0 -> NEG), in score layout ----
                KVAR = os.environ.get("KVAR", "")
                masks = []
                for s in range(S if stage >= 1 else 0):
                    msk = spool.tile([128, NCOL], F32, name=f"mask{s}", tag=f"mask{s}")
                    if "nomaskdma" in KVAR:
                        nc.vector.memset(msk, 0.0)
                    else:
                        msrc = scr.tile([128, NCOL], F32, tag="msrc")
                        nc.scalar.dma_start(
                            msrc, d_S[s].rearrange("(c p j) -> p c j", p=128, j=JPB))
                        nc.vector.tensor_scalar(msk, msrc, 0.0, NEG,
                                                op0=OP.is_le, op1=OP.mult)
                    masks.append(msk)

                # ---- scoring: chunked DMA + segmented-dot scan (custom DVE) ----
                # One DVE instruction per chunk: prefix-sum of K*q over the
                # whole [128, JPB*DE] stream; a stride-0 innermost out AP keeps
                # only the prefix at each page end. Adjacent-difference then
                # yields the per-slot dot products.
                PJ = JPB + 1  # prefix columns per chunk (col 0 stays 0)
                scores = [spool.tile([128, NCOL], F32, name=f"scores{s}", tag=f"scores{s}")
                          for s in range(S)]
                pcols = []
                for s in range(S if stage >= 1 else 0):
                    pcol = spool.tile([128, NCHUNK * PJ], F32, name=f"pcol{s}",
                                      tag=f"pcol{s}")
                    nc.vector.memset(pcol, 0.0)
                    pcols.append(pcol)
                for s in range(S if stage >= 1 else 0):
                    for c in range(NCHUNK):
                        kt = kpool.tile([128, CH], F32, tag="ktile")
                        base = s * M + c * CH
                        # issue on the (otherwise idle) gpsimd ring: K DMAs
                        # then start at t=0 instead of queueing behind the
                        # query phase's semaphores on the sync ring
                        nc.gpsimd.dma_start(
                            kt, d_K[base:base + CH, :].rearrange(
                                "(p j) d -> p j d", p=128))
                        if "noscore" in KVAR:
                            continue
                        in0 = kt.rearrange("p (j d) -> p j d", d=DE)
                        in1 = q_rep[s].unsqueeze(1).broadcast_to([128, JPB, DE])
                        out3 = pcols[s][:, c * PJ + 1:c * PJ + 1 + JPB].unsqueeze(
                            2).broadcast_to([128, JPB, DE])
                        nc.vector._custom_dve(dot_op, out=out3, in0=in0, in1=in1)
                    if "noscore" in KVAR:
                        nc.vector.memset(scores[s], 0.0)
                        continue
                    # scores = prefix[j+1] - prefix[j], then mask add
                    p3 = pcols[s].rearrange("p (c j) -> p c j", j=PJ)
                    sc3 = scores[s].rearrange("p (c j) -> p c j", j=JPB)
                    nc.vector.tensor_sub(sc3, p3[:, :, 1:PJ], p3[:, :, 0:JPB])
                    nc.vector.tensor_add(scores[s], scores[s], masks[s])

                if stage == 1:
                    for s in range(S):
                        nc.sync.dma_start(
                            d_out[s:s + 1, :].rearrange("one (p r) -> p one r", p=128),
                            scores[s][:, :8])

                if stage >= 2:
                    # ---- selection stage 1: per-partition top-8 ----
                    cand = spool.tile([S, NCAND], F32, name="cand", tag="cand")
                    for s in range(S):
                        v8 = small.tile([128, 8], F32, tag="v8")
                        nc.vector.max(out=v8, in_=scores[s])
                        c8 = small.tile([128, 8], U32, tag="c8")
                        nc.vector.max_index(out=c8, in_max=v8, in_values=scores[s])
                        # em row = s*M + (c8>>log2(JPB))*CH + p*JPB + (c8&(JPB-1))
                        jb = int(np.log2(JPB))
                        t1 = small.tile([128, 8], U32, tag="t1")
                        nc.vector.tensor_scalar(t1, c8, jb, None,
                                                op0=OP.arith_shift_right)
                        t1b = small.tile([128, 8], U32, tag="t1b")
                        nc.vector.tensor_scalar(t1b, t1, CH, s * M,
                                                op0=OP.mult, op1=OP.add)
                        t2 = small.tile([128, 8], U32, tag="t2")
                        nc.vector.tensor_scalar(t2, c8, JPB - 1, None,
                                                op0=OP.bitwise_and)
                        t3 = small.tile([128, 8], U32, tag="t3")
                        nc.vector.tensor_add(t3, t1b, t2)
                        gidx = small.tile([128, 8], U32, tag="gidx")
                        nc.vector.tensor_add(gidx, t3, iota32.to_broadcast([128, 8]))
                        # stash values + index table
                        nc.sync.dma_start(cand[s:s + 1, :], v8)
                        nc.sync.dma_start(
                            d_gtab[s * NCAND:(s + 1) * NCAND, :].rearrange(
                                "(p r) one -> p r one", p=128), gidx)

                    # ---- selection stage 2: fold 1024 -> top-32 per stream ----
                    tv = spool.tile([S, KRET], F32, name="tv", tag="tv")
                    tc_ = spool.tile([S, KRET], U32, name="tc", tag="tc")
                    for r in range(4):
                        sl = slice(8 * r, 8 * r + 8)
                        nc.vector.max(out=tv[:, sl], in_=cand)
                        nc.vector.max_index(out=tc_[:, sl], in_max=tv[:, sl],
                                            in_values=cand)
                        if r < 3:
                            nc.vector.match_replace(out=cand, in_to_replace=tv[:, sl],
                                                    in_values=cand, imm_value=NEG)
                    tcg = spool.tile([S, KRET], F32, name="tcg", tag="tcg")
                    nc.vector.tensor_add(tcg, tc_, iotaS.to_broadcast([S, KRET]))

                    # transpose tv/tcg -> columns [KRET, S]
                    tcT_ps = pp.tile([128, S], F32, space="PSUM", tag="tr")
                    nc.tensor.transpose(tcT_ps[:KRET, :], tcg, ident[:S, :S])
                    tcT = spool.tile([KRET, S], I32, name="tcT", tag="tcT")
                    nc.vector.tensor_copy(tcT, tcT_ps[:KRET, :])
                    tvT_ps = pp.tile([128, S], F32, space="PSUM", tag="tr")
                    nc.tensor.transpose(tvT_ps[:KRET, :], tv, ident[:S, :S])
                    tvT = spool.tile([KRET, S], F32, name="tvT", tag="tvT")
                    nc.vector.tensor_copy(tvT, tvT_ps[:KRET, :])

                    # chained gathers (per stream): index table, then em_V rows
                    gsel = small.tile([KRET, S], U32, tag="gsel")
                    for s in range(S):
                        nc.gpsimd.indirect_dma_start(
                            out=gsel[:, s:s + 1], out_offset=None, in_=d_gtab,
                            in_offset=IndirectOffsetOnAxis(
                                ap=tcT[:, s:s + 1], axis=0))
                    gseli = small.tile([KRET, S], I32, tag="gseli")
                    nc.vector.tensor_copy(gseli, gsel)
                    vtop4 = spool.tile([KRET, S * DE], F32, name="vtop4",
                                       tag="vtop4")
                    for s in range(S):
                        nc.gpsimd.indirect_dma_start(
                            out=vtop4[:, s * DE:(s + 1) * DE], out_offset=None,
                            in_=d_V,
                            in_offset=IndirectOffsetOnAxis(
                                ap=gseli[:, s:s + 1], axis=0))

                if stage == 2:
                    for s in range(S):
                        nc.sync.dma_start(
                            d_out[s:s + 1, :].rearrange(
                                "one (p r) -> p one r", p=KRET),
                            vtop4[:, s * DE:s * DE + KRET])

                if stage >= 3:
                    # ---- phase A (batched): attention + softmax ----
                    attn4 = small.tile([KRET, S], F32, tag="attn4")
                    for s in range(S):
                        prodA = scr.tile([KRET, 1], F32, tag="prodA")
                        nc.vector.scalar_tensor_tensor(
                            out=prodA.broadcast_to([KRET, DE]),
                            in0=vtop4[:, s * DE:(s + 1) * DE],
                            scalar=float(DE ** -0.5), in1=qc_rep[s],
                            op0=OP.mult, op1=OP.mult,
                            accum_out=attn4[:, s:s + 1])
                    nc.vector.tensor_add(attn4, attn4, tvT)
                    aT_ps = pp.tile([128, KRET], F32, space="PSUM", tag="tr")
                    nc.tensor.transpose(aT_ps[:S, :], attn4, ident[:KRET, :KRET])
                    aT = small.tile([S, KRET], F32, tag="aT")
                    nc.vector.tensor_copy(aT, aT_ps[:S, :])
                    mx4 = small.tile([S, 1], F32, tag="mx4")
                    nc.vector.reduce_max(mx4, aT, axis=mybir.AxisListType.X)
                    nc.vector.tensor_scalar(aT, aT, mx4, None, op0=OP.subtract)
                    ew = small.tile([S, KRET], F32, tag="ew")
                    sume4 = small.tile([S, 1], F32, tag="sume4")
                    nc.scalar.activation(ew, aT, AF.Exp, accum_out=sume4)
                    rcp4 = small.tile([S, 1], F32, tag="rcp4")
                    nc.vector.reciprocal(rcp4, sume4)
                    nc.vector.tensor_scalar(ew, ew, rcp4, None, op0=OP.mult)
                    wT_ps = pp.tile([128, S], F32, space="PSUM", tag="tr")
                    nc.tensor.transpose(wT_ps[:KRET, :], ew, ident[:S, :S])
                    wT = small.tile([KRET, S], F32, tag="wT")
                    nc.vector.tensor_copy(wT, wT_ps[:KRET, :])
                    ps_oe = pacc.tile([128, S], F32, space="PSUM", tag="acc")
                    for s in range(S):
                        nc.tensor.matmul(ps_oe[:, s:s + 1],
                                         lhsT=vtop4[:, s * DE:(s + 1) * DE],
                                         rhs=wT[:, s:s + 1], start=True, stop=True)
                    h04 = spool.tile([128, S], F32, name="h04", tag="h04")
                    nc.vector.tensor_copy(h04, ps_oe)

                    # ---- phase B (batched): layernorm + FFN + out proj ----
                    ps_s1 = pq.tile([S, 1], F32, space="PSUM", tag="row")
                    nc.tensor.matmul(ps_s1, lhsT=h04, rhs=ones_col,
                                     start=True, stop=True)
                    mean4 = small.tile([S, 1], F32, tag="mean4")
                    nc.vector.tensor_scalar(mean4, ps_s1, 1.0 / DE, None,
                                            op0=OP.mult)
                    mr_ps = pp.tile([128, S], F32, space="PSUM", tag="tr")
                    nc.tensor.transpose(mr_ps[:1, :], mean4, ident[:S, :S])
                    mrow = small.tile([1, S], F32, tag="mrow")
                    nc.vector.tensor_copy(mrow, mr_ps[:1, :])
                    mb_ps = pp.tile([128, S], F32, space="PSUM", tag="tr")
                    nc.tensor.matmul(mb_ps, lhsT=ones_row, rhs=mrow,
                                     start=True, stop=True)
                    c4 = small.tile([128, S], F32, tag="c4")
                    nc.vector.tensor_sub(c4, h04, mb_ps)
                    ps_vv = pq.tile([S, S], F32, space="PSUM", tag="row")
                    nc.tensor.matmul(ps_vv, lhsT=c4, rhs=c4, start=True, stop=True)
                    vd = small.tile([S, S], F32, tag="vd")
                    nc.vector.tensor_mul(vd, ps_vv, ident[:S, :S])
                    var4 = small.tile([S, 1], F32, tag="var4")
                    nc.vector.reduce_sum(var4, vd, axis=mybir.AxisListType.X)
                    nc.vector.tensor_scalar(var4, var4, 1.0 / DE, None, op0=OP.mult)
                    lnv = small.tile([S, 1], F32, tag="lnv")
                    nc.scalar.activation(lnv, var4, AF.Ln, bias=eps5[:S, :])
                    nc.vector.tensor_scalar(lnv, lnv, -0.5, None, op0=OP.mult)
                    rstd4 = small.tile([S, 1], F32, tag="rstd4")
                    nc.scalar.activation(rstd4, lnv, AF.Exp)
                    rr_ps = pp.tile([128, S], F32, space="PSUM", tag="tr")
                    nc.tensor.transpose(rr_ps[:1, :], rstd4, ident[:S, :S])
                    rrow = small.tile([1, S], F32, tag="rrow")
                    nc.vector.tensor_copy(rrow, rr_ps[:1, :])
                    rb_ps = pp.tile([128, S], F32, space="PSUM", tag="tr")
                    nc.tensor.matmul(rb_ps, lhsT=ones_row, rhs=rrow,
                                     start=True, stop=True)
                    hln4 = small.tile([128, S], F32, tag="hln4")
                    nc.vector.tensor_mul(hln4, c4, rb_ps)
                    nc.vector.tensor_mul(hln4, hln4, lng_c.to_broadcast([128, S]))
                    nc.vector.tensor_add(hln4, hln4, lnb_c.to_broadcast([128, S]))

                    ps_h1 = pacc.tile([128, 4 * S], F32, space="PSUM", tag="acc")
                    for k in range(4):
                        nc.tensor.matmul(ps_h1[:, k * S:(k + 1) * S],
                                         lhsT=w1[:, k * 128:(k + 1) * 128],
                                         rhs=hln4, start=True, stop=True)
                    t14 = small.tile([128, 4 * S], F32, tag="t14")
                    for k in range(4):
                        nc.vector.tensor_add(t14[:, k * S:(k + 1) * S],
                                             ps_h1[:, k * S:(k + 1) * S],
                                             b1_c[:, k:k + 1].to_broadcast([128, S]))
                    g14 = small.tile([128, 4 * S], F32, tag="g14")
                    nc.scalar.activation(g14, t14, act_fn)

                    ps_h2 = pacc.tile([128, S], F32, space="PSUM", tag="acc")
                    for k in range(4):
                        nc.tensor.matmul(ps_h2, lhsT=w2[:, k * DE:(k + 1) * DE],
                                         rhs=g14[:, k * S:(k + 1) * S],
                                         start=(k == 0), stop=(k == 3))
                    r4 = small.tile([128, S], F32, tag="r4")
                    nc.vector.tensor_add(r4, ps_h2, b2_c.to_broadcast([128, S]))
                    nc.vector.tensor_add(r4, r4, h04)
                    y4 = small.tile([S, D], F32, tag="y4")
                    for k in range(2):
                        ps_y = pq.tile([S, 512], F32, space="PSUM", tag="row")
                        nc.tensor.matmul(ps_y, lhsT=r4,
                                         rhs=wo[:, k * 512:(k + 1) * 512],
                                         start=True, stop=True)
                        nc.vector.tensor_add(y4[:, k * 512:(k + 1) * 512], ps_y,
                                             bo4[:, k * 512:(k + 1) * 512])
                    nc.sync.dma_start(d_out, y4)

    nc.compile()
    return nc


_NC_CACHE = {}


def _get_nc(M=32768, debug=False, stage=99):
    key = (M, debug, stage)
    if key not in _NC_CACHE:
        _NC_CACHE[key] = build_nc(M=M, debug=debug, stage=stage)
    return _NC_CACHE[key]


def make_in_maps(inputs, M=32768, ncores=NCORES):
    """Split full inputs into per-core input maps."""
    JPB = min(4096, M) // 128
    shared = {
        "cst_ident": np.eye(128, dtype=np.float32),
        "cst_iota_jpb": (np.arange(128, dtype=np.uint32) * JPB)[:, None],
        "cst_iota_s": (np.arange(S, dtype=np.uint32) * 1024)[:, None],
    }
    for name in ["Wq_em_w", "Wq_em_b", "Wq_cross_w", "Wq_cross_b", "Wo_w",
                 "Wo_b", "ln_g", "ln_b", "ffn1_w", "ffn1_b", "ffn2_w", "ffn2_b"]:
        shared[name] = np.ascontiguousarray(np.asarray(inputs[name], np.float32))
    in_maps = []
    for c in range(ncores):
        sl = slice(c * S, (c + 1) * S)
        m = dict(shared)
        m["x"] = np.ascontiguousarray(np.asarray(inputs["x"][sl], np.float32))
        m["y_wm"] = np.ascontiguousarray(np.asarray(inputs["y_wm"][sl], np.float32))
        m["em_K"] = np.ascontiguousarray(
            np.asarray(inputs["em_K"][sl], np.float32).reshape(S * M, DE))
        m["em_V"] = np.ascontiguousarray(
            np.asarray(inputs["em_V"][sl], np.float32).reshape(S * M, DE))
        m["em_S"] = np.ascontiguousarray(np.asarray(inputs["em_S"][sl], np.float32))
        in_maps.append(m)
    return in_maps


def kernel(**inputs):
    from concourse.bass_utils import run_bass_kernel_spmd

    nc = _get_nc()
    in_maps = make_in_maps(inputs)
    res = run_bass_kernel_spmd(nc, in_maps, list(range(NCORES))).results
    return np.concatenate([res[c]["out"] for c in range(NCORES)], axis=0)



# revision 12
# speedup vs baseline: 1.1951x; 1.1951x over previous
"""Trainium2 Bass kernel for nn_EpisodicMemory (scatter_memory).

Sharding: pure batch data-parallelism. 8 cores, 32 streams -> 4 streams/core.
Each core runs the full per-stream pipeline:
  q projections (PE) -> masked cosine scores over M=32768 slots (DVE
  tensor_tensor_reduce, em_K consumed in natural [slot, d] layout, em_S mask
  folded in as the reduce init scalar) -> per-partition top-8 (DVE Max8) ->
  batched fold to top-32 -> chained indirect DMA gathers (index table, em_V
  rows) -> cross-attention + softmax + FFN epilogue (PE/ACT, tiny).

`stage` (debug): 1 = scoring only (dump scores), 2 = + selection/gather
(dump V_top), 99 = full.
"""

import os
import sys

import numpy as np

sys.path.insert(0, "/opt/trn_rl_repo")

import concourse.bass as bass  # noqa: F401
import concourse.tile as tile
from concourse import bacc, mybir
from concourse.bass import IndirectOffsetOnAxis
from concourse.masks import make_identity

F32 = mybir.dt.float32
I32 = mybir.dt.int32
U32 = mybir.dt.uint32
OP = mybir.AluOpType
AF = mybir.ActivationFunctionType

NCORES = 8
BS, D, DE, KRET = 32, 1024, 128, 32
S = BS // NCORES  # streams per core = 4
NEG = -3.0e30  # stand-in for -inf (safe for exp/compare, no NaNs)


def register_dot_prefix():
    """Custom DVE op: out = running prefix-sum of Src0*Src1 along the free
    stream. With a stride-0 innermost out AP, the surviving write per page
    is the prefix total at that page's end -> segmented dot products in one
    instruction per chunk (vs one scalar_tensor_tensor + accum-read per
    128-slot column)."""
    from concourse.dve_ops import (
        CUSTOM_DVE_SPECS,
        OPS,
        _CUSTOM_DVE_ROW_BASE,
        _SUB_OPCODE_FOR_NAME,
        DveOp,
    )
    from concourse.dve_spec import AluOp, Spec, Src0, Src1, lower, scan
    from concourse.dve_uop import DveOpSpec

    name = "DOT_PREFIX_ANT"
    if name in _SUB_OPCODE_FOR_NAME:
        return next(op for op in OPS if op.name == name)

    def _ref(in0, in1, s0, s1, imm2):
        p = in0.shape[0]
        a = np.asarray(in0, np.float32).reshape(p, -1)
        b = np.asarray(in1, np.float32).reshape(p, -1)
        return np.cumsum(a * b, axis=-1, dtype=np.float32).reshape(in0.shape)

    spec = Spec(body=scan(AluOp.ADD, Src0 * Src1), reference=_ref)
    row = _CUSTOM_DVE_ROW_BASE + len(OPS)
    sha = {}
    for ver in ("v3", "v4"):
        tmp = DveOpSpec(name=name, opcode=row, uops=lower(spec, ver=ver), rd1_en=True)
        sha[ver] = tmp.sha(ver)
    op = DveOp(name, spec, subdim=False, uops_sha=sha)
    OPS.append(op)
    CUSTOM_DVE_SPECS[name] = spec
    _SUB_OPCODE_FOR_NAME[name] = row
    return op


def build_nc(M=32768, debug=False, act_fn=None, stage=99, reps=1, serial_reps=False):
    """Build the per-core Bass kernel. M = slots per stream (param for sim)."""
    if act_fn is None:
        act_fn = AF.Gelu
    CH = min(4096, M)         # slots per DMA chunk (4096 slots = 2 MB)
    NCHUNK = M // CH
    JPB = CH // 128           # rows per partition per chunk (32)
    NCOL = M // 128           # score columns (256)
    NCAND = 1024              # per-stream candidates (128 partitions x 8)

    dot_op = register_dot_prefix()
    nc = bacc.Bacc("TRN2", target_bir_lowering=False, debug=debug)

    # ---- DRAM I/O (per-core shard) ----
    d_x = nc.dram_tensor("x", [S, D], F32, kind="ExternalInput").ap()
    d_y = nc.dram_tensor("y_wm", [S, D], F32, kind="ExternalInput").ap()
    d_K = nc.dram_tensor("em_K", [S * M, DE], F32, kind="ExternalInput").ap()
    d_V = nc.dram_tensor("em_V", [S * M, DE], F32, kind="ExternalInput").ap()
    d_S = nc.dram_tensor("em_S", [S, M], F32, kind="ExternalInput").ap()
    d_wqe = nc.dram_tensor("Wq_em_w", [2 * D, DE], F32, kind="ExternalInput").ap()
    d_bqe = nc.dram_tensor("Wq_em_b", [DE], F32, kind="ExternalInput").ap()
    d_wqc = nc.dram_tensor("Wq_cross_w", [D, DE], F32, kind="ExternalInput").ap()
    d_bqc = nc.dram_tensor("Wq_cross_b", [DE], F32, kind="ExternalInput").ap()
    d_wo = nc.dram_tensor("Wo_w", [DE, D], F32, kind="ExternalInput").ap()
    d_bo = nc.dram_tensor("Wo_b", [D], F32, kind="ExternalInput").ap()
    d_lng = nc.dram_tensor("ln_g", [DE], F32, kind="ExternalInput").ap()
    d_lnb = nc.dram_tensor("ln_b", [DE], F32, kind="ExternalInput").ap()
    d_w1 = nc.dram_tensor("ffn1_w", [DE, 4 * DE], F32, kind="ExternalInput").ap()
    d_b1 = nc.dram_tensor("ffn1_b", [4 * DE], F32, kind="ExternalInput").ap()
    d_w2 = nc.dram_tensor("ffn2_w", [4 * DE, DE], F32, kind="ExternalInput").ap()
    d_b2 = nc.dram_tensor("ffn2_b", [DE], F32, kind="ExternalInput").ap()
    d_out = nc.dram_tensor("out", [S, D], F32, kind="ExternalOutput").ap()
    d_ident = nc.dram_tensor("cst_ident", [128, 128], F32, kind="ExternalInput").ap()
    d_iotaj = nc.dram_tensor("cst_iota_jpb", [128, 1], U32, kind="ExternalInput").ap()
    d_iotas = nc.dram_tensor("cst_iota_s", [S, 1], U32, kind="ExternalInput").ap()
    # index table for the chained gather (slot row ids as uint32)
    d_gtab = nc.dram_tensor("gtab", [S * NCAND, 1], U32).ap()

    with tile.TileContext(nc) as tc:
        with (
            tc.tile_pool(name="kpool", bufs=7) as kpool,
            tc.tile_pool(name="wpool", bufs=1) as wpool,
            tc.tile_pool(name="spool", bufs=1) as spool,
            tc.tile_pool(name="scr", bufs=2) as scr,
            tc.tile_pool(name="small", bufs=4) as small,
            tc.tile_pool(name="pp", bufs=3, space="PSUM") as pp,
            tc.tile_pool(name="pacc", bufs=2, space="PSUM") as pacc,
            tc.tile_pool(name="pq", bufs=2, space="PSUM") as pq,
        ):
            # ---- constants / weights in SBUF ----
            ident = wpool.tile([128, 128], F32, name="ident")
            nc.sync.dma_start(ident, d_ident)
            ones_row = wpool.tile([1, 128], F32, name="ones_row")
            nc.vector.memset(ones_row, 1.0)
            ones_col = wpool.tile([128, 1], F32, name="ones_col")
            nc.vector.memset(ones_col, 1.0)
            iota32 = wpool.tile([128, 1], U32, name="iota32")  # p * JPB
            nc.sync.dma_start(iota32, d_iotaj)
            iotaS = wpool.tile([S, 1], U32, name="iotaS")  # s * NCAND
            nc.sync.dma_start(iotaS, d_iotas)
            eps12 = wpool.tile([128, 1], F32, name="eps12")
            nc.vector.memset(eps12, 1e-12)
            eps5 = wpool.tile([128, 1], F32, name="eps5")
            nc.vector.memset(eps5, 1e-5)

            # Wq_em rows 2048 -> [128, 16*128]; Wq_cross rows 1024 -> [128, 8*128]
            wqe = wpool.tile([128, 16 * DE], F32, name="wqe")
            nc.sync.dma_start(wqe, d_wqe.rearrange("(j p) e -> p j e", p=128))
            wqc = wpool.tile([128, 8 * DE], F32, name="wqc")
            nc.sync.dma_start(wqc, d_wqc.rearrange("(j p) e -> p j e", p=128))
            w1 = wpool.tile([128, 512], F32, name="w1")
            nc.sync.dma_start(w1, d_w1)
            w2 = wpool.tile([128, 4 * DE], F32, name="w2")
            nc.sync.dma_start(w2, d_w2.rearrange("(k p) e -> p k e", p=128))
            wo = wpool.tile([128, D], F32, name="wo")
            nc.sync.dma_start(wo, d_wo)
            bqe_c = wpool.tile([128, 1], F32, name="bqe_c")
            nc.sync.dma_start(bqe_c, d_bqe[:, None])
            bqc_c = wpool.tile([128, 1], F32, name="bqc_c")
            nc.sync.dma_start(bqc_c, d_bqc[:, None])
            lng_c = wpool.tile([128, 1], F32, name="lng_c")
            nc.sync.dma_start(lng_c, d_lng[:, None])
            lnb_c = wpool.tile([128, 1], F32, name="lnb_c")
            nc.sync.dma_start(lnb_c, d_lnb[:, None])
            b1_c = wpool.tile([128, 4], F32, name="b1_c")
            nc.sync.dma_start(b1_c, d_b1.rearrange("(k p) -> p k", p=128))
            b2_c = wpool.tile([128, 1], F32, name="b2_c")
            nc.sync.dma_start(b2_c, d_b2[:, None])
            bo4 = wpool.tile([S, D], F32, name="bo4")
            for _s in range(S):
                nc.sync.dma_start(bo4[_s:_s + 1, :], d_bo[None, :])

            def bcast_col(val11, n=128):
                """[1,1] sbuf -> [n,1] sbuf via PE outer product."""
                ps = pp.tile([128, 1], F32, space="PSUM", tag="tr")
                nc.tensor.matmul(ps[:n, :], lhsT=ones_row[:, :n], rhs=val11,
                                 start=True, stop=True)
                sb = small.tile([n, 1], F32, tag="bc_sb")
                nc.vector.tensor_copy(sb, ps[:n, :])
                return sb

            def transpose(src, pdim, fdim):
                """[pdim, fdim] -> psum [fdim, pdim]; returns psum AP."""
                ps = pp.tile([128, 128], F32, space="PSUM", tag="tr")
                nc.tensor.transpose(ps[:fdim, :pdim], src, ident[:pdim, :pdim])
                return ps[:fdim, :pdim]

            def rsqrt11(val11, eps_ap, tag):
                """[1,1] -> 1/sqrt(val + eps) via exp(-0.5 * ln(val + eps))."""
                t = small.tile([1, 1], F32, tag=tag + "_ln")
                nc.scalar.activation(t, val11, AF.Ln, bias=eps_ap)
                t2 = small.tile([1, 1], F32, tag=tag + "_sc")
                nc.vector.tensor_scalar(t2, t, -0.5, None, op0=OP.mult)
                r = small.tile([1, 1], F32, tag=tag + "_ex")
                nc.scalar.activation(r, t2, AF.Exp)
                return r

            for rep_ in range(reps):
                if serial_reps and rep_ > 0:
                    fence = scr.tile([S, D], F32, tag="fence")
                    nc.sync.dma_start(fence, d_out)
                    fs = small.tile([S, 1], F32, tag="fs")
                    nc.vector.reduce_max(fs, fence, axis=mybir.AxisListType.X)
                # ---- prefetch: first kpool-bufs K chunks, issued on the
                # scalar engine's HW ring as its first instructions so the
                # DMA engines are saturated from t=0 (the sync ring is
                # blocked behind the query phase's semaphores) ----
                PREF = 7
                kt_pre = []
                if stage >= 1:
                    sc_pairs = [(s, c) for s in range(S) for c in range(NCHUNK)]
                    for (s, c) in sc_pairs[:PREF]:
                        kt = kpool.tile([128, CH], F32, tag="ktile")
                        base = s * M + c * CH
                        nc.scalar.dma_start(
                            kt, d_K[base:base + CH, :].rearrange(
                                "(p j) d -> p j d", p=128))
                        kt_pre.append(kt)

                # ---- phase 0: per-stream queries ----
                q_rep, qc_rep = [], []
                for s in range(S):
                    xn = scr.tile([8, 128], F32, tag="xn")
                    nc.sync.dma_start(xn, d_x[s].rearrange("(j d) -> j d", j=8))
                    yn = scr.tile([8, 128], F32, tag="yn")
                    nc.sync.dma_start(yn, d_y[s].rearrange("(j d) -> j d", j=8))
                    xT_ps = transpose(xn, 8, 128)
                    xT = scr.tile([128, 8], F32, tag="xT")
                    nc.vector.tensor_copy(xT, xT_ps)
                    yT_ps = transpose(yn, 8, 128)
                    yT = scr.tile([128, 8], F32, tag="yT")
                    nc.vector.tensor_copy(yT, yT_ps)

                    # q_em = normalize(concat(x,y) @ Wq_em + b)
                    ps_q = pacc.tile([128, 1], F32, space="PSUM", tag="acc")
                    for j in range(16):
                        rhs = xT[:, j:j + 1] if j < 8 else yT[:, j - 8:j - 7]
                        nc.tensor.matmul(ps_q, lhsT=wqe[:, j * DE:(j + 1) * DE], rhs=rhs,
                                         start=(j == 0), stop=(j == 15))
                    q_sb = small.tile([128, 1], F32, tag="q_sb")
                    nc.vector.tensor_add(q_sb, ps_q, bqe_c)
                    ps_n = pq.tile([1, 1], F32, space="PSUM", tag="row")
                    nc.tensor.matmul(ps_n, lhsT=q_sb, rhs=q_sb, start=True, stop=True)
                    n2 = small.tile([1, 1], F32, tag="n2")
                    nc.vector.tensor_copy(n2, ps_n)
                    inv = rsqrt11(n2, eps12[:1, :], tag="qinv")
                    inv_b = bcast_col(inv)
                    qhat = small.tile([128, 1], F32, tag="qhat")
                    nc.vector.tensor_mul(qhat, q_sb, inv_b)
                    qrow_ps = transpose(qhat, 128, 1)
                    qrow = small.tile([1, 128], F32, tag="qrow")
                    nc.vector.tensor_copy(qrow, qrow_ps)
                    ps_qr = pp.tile([128, 128], F32, space="PSUM", tag="tr")
                    nc.tensor.matmul(ps_qr, lhsT=ones_row, rhs=qrow, start=True, stop=True)
                    qr = spool.tile([128, 128], F32, name=f"q_rep{s}", tag=f"q_rep{s}")
                    nc.vector.tensor_copy(qr, ps_qr)
                    q_rep.append(qr)

                    # q_cross = x @ Wq_cross + b
                    ps_qc = pacc.tile([128, 1], F32, space="PSUM", tag="acc")
                    for j in range(8):
                        nc.tensor.matmul(ps_qc, lhsT=wqc[:, j * DE:(j + 1) * DE],
                                         rhs=xT[:, j:j + 1], start=(j == 0), stop=(j == 7))
                    qc_sb = small.tile([128, 1], F32, tag="qc_sb")
                    nc.vector.tensor_add(qc_sb, ps_qc, bqc_c)
                    qcrow_ps = transpose(qc_sb, 128, 1)
                    qcrow = small.tile([1, 128], F32, tag="qcrow")
                    nc.vector.tensor_copy(qcrow, qcrow_ps)
                    ps_qcr = pp.tile([128, 128], F32, space="PSUM", tag="tr")
                    nc.tensor.matmul(ps_qcr[:KRET, :], lhsT=ones_row[:, :KRET], rhs=qcrow,
                                     start=True, stop=True)
                    qcr = spool.tile([KRET, 128], F32, name=f"qc_rep{s}", tag=f"qc_rep{s}")
                    nc.vector.tensor_copy(qcr, ps_qcr[:KRET, :])
                    qc_rep.append(qcr)

                if stage == 0:
                    for s in range(S):
                        nc.sync.dma_start(
                            d_out[s:s + 1, :].rearrange("one (p r) -> p one r", p=128),
                            q_rep[s][:, :8])

                # ---- masks (em_S <= 0 -> NEG), in score layout ----
                KVAR = os.environ.get("KVAR", "")
                masks = []
                for s in range(S if stage >= 1 else 0):
                    msk = spool.tile([128, NCOL], F32, name=f"mask{s}", tag=f"mask{s}")
                    if "nomaskdma" in KVAR:
                        nc.vector.memset(msk, 0.0)
                    else:
                        msrc = scr.tile([128, NCOL], F32, tag="msrc")
                        nc.sync.dma_start(
                            msrc, d_S[s].rearrange("(c p j) -> p c j", p=128, j=JPB))
                        nc.vector.tensor_scalar(msk, msrc, 0.0, NEG,
                                                op0=OP.is_le, op1=OP.mult)
                    masks.append(msk)

                # ---- scoring: chunked DMA + segmented-dot scan (custom DVE) ----
                # One DVE instruction per chunk: prefix-sum of K*q over the
                # whole [128, JPB*DE] stream; a stride-0 innermost out AP keeps
                # only the prefix at each page end. Adjacent-difference then
                # yields the per-slot dot products.
                PJ = JPB + 1  # prefix columns per chunk (col 0 stays 0)
                scores = [spool.tile([128, NCOL], F32, name=f"scores{s}", tag=f"scores{s}")
                          for s in range(S)]
                pcols = []
                for s in range(S if stage >= 1 else 0):
                    pcol = spool.tile([128, NCHUNK * PJ], F32, name=f"pcol{s}",
                                      tag=f"pcol{s}")
                    nc.vector.memset(pcol, 0.0)
                    pcols.append(pcol)
                for s in range(S if stage >= 1 else 0):
                    for c in range(NCHUNK):
                        sc_idx = s * NCHUNK + c
                        if sc_idx < len(kt_pre):
                            kt = kt_pre[sc_idx]
                        else:
                            kt = kpool.tile([128, CH], F32, tag="ktile")
                            base = s * M + c * CH
                            nc.scalar.dma_start(
                                kt, d_K[base:base + CH, :].rearrange(
                                    "(p j) d -> p j d", p=128))
                        if "noscore" in KVAR:
                            continue
                        in0 = kt.rearrange("p (j d) -> p j d", d=DE)
                        in1 = q_rep[s].unsqueeze(1).broadcast_to([128, JPB, DE])
                        out3 = pcols[s][:, c * PJ + 1:c * PJ + 1 + JPB].unsqueeze(
                            2).broadcast_to([128, JPB, DE])
                        nc.vector._custom_dve(dot_op, out=out3, in0=in0, in1=in1)
                    if "noscore" in KVAR:
                        nc.vector.memset(scores[s], 0.0)
                        continue
                    # scores = prefix[j+1] - prefix[j], then mask add
                    p3 = pcols[s].rearrange("p (c j) -> p c j", j=PJ)
                    sc3 = scores[s].rearrange("p (c j) -> p c j", j=JPB)
                    nc.vector.tensor_sub(sc3, p3[:, :, 1:PJ], p3[:, :, 0:JPB])
                    nc.vector.tensor_add(scores[s], scores[s], masks[s])

                if stage == 1:
                    for s in range(S):
                        nc.sync.dma_start(
                            d_out[s:s + 1, :].rearrange("one (p r) -> p one r", p=128),
                            scores[s][:, :8])

                if stage >= 2:
                    # ---- selection stage 1: per-partition top-8 ----
                    cand = spool.tile([S, NCAND], F32, name="cand", tag="cand")
                    for s in range(S):
                        v8 = small.tile([128, 8], F32, tag="v8")
                        nc.vector.max(out=v8, in_=scores[s])
                        c8 = small.tile([128, 8], U32, tag="c8")
                        nc.vector.max_index(out=c8, in_max=v8, in_values=scores[s])
                        # em row = s*M + (c8>>log2(JPB))*CH + p*JPB + (c8&(JPB-1))
                        jb = int(np.log2(JPB))
                        t1 = small.tile([128, 8], U32, tag="t1")
                        nc.vector.tensor_scalar(t1, c8, jb, None,
                                                op0=OP.arith_shift_right)
                        t1b = small.tile([128, 8], U32, tag="t1b")
                        nc.vector.tensor_scalar(t1b, t1, CH, s * M,
                                                op0=OP.mult, op1=OP.add)
                        t2 = small.tile([128, 8], U32, tag="t2")
                        nc.vector.tensor_scalar(t2, c8, JPB - 1, None,
                                                op0=OP.bitwise_and)
                        t3 = small.tile([128, 8], U32, tag="t3")
                        nc.vector.tensor_add(t3, t1b, t2)
                        gidx = small.tile([128, 8], U32, tag="gidx")
                        nc.vector.tensor_add(gidx, t3, iota32.to_broadcast([128, 8]))
                        # stash values + index table
                        nc.sync.dma_start(cand[s:s + 1, :], v8)
                        nc.sync.dma_start(
                            d_gtab[s * NCAND:(s + 1) * NCAND, :].rearrange(
                                "(p r) one -> p r one", p=128), gidx)

                    # ---- selection stage 2: fold 1024 -> top-32 per stream ----
                    tv = spool.tile([S, KRET], F32, name="tv", tag="tv")
                    tc_ = spool.tile([S, KRET], U32, name="tc", tag="tc")
                    for r in range(4):
                        sl = slice(8 * r, 8 * r + 8)
                        nc.vector.max(out=tv[:, sl], in_=cand)
                        nc.vector.max_index(out=tc_[:, sl], in_max=tv[:, sl],
                                            in_values=cand)
                        if r < 3:
                            nc.vector.match_replace(out=cand, in_to_replace=tv[:, sl],
                                                    in_values=cand, imm_value=NEG)
                    tcg = spool.tile([S, KRET], F32, name="tcg", tag="tcg")
                    nc.vector.tensor_add(tcg, tc_, iotaS.to_broadcast([S, KRET]))

                    # transpose tv/tcg -> columns [KRET, S]
                    tcT_ps = pp.tile([128, S], F32, space="PSUM", tag="tr")
                    nc.tensor.transpose(tcT_ps[:KRET, :], tcg, ident[:S, :S])
                    tcT = spool.tile([KRET, S], I32, name="tcT", tag="tcT")
                    nc.vector.tensor_copy(tcT, tcT_ps[:KRET, :])
                    tvT_ps = pp.tile([128, S], F32, space="PSUM", tag="tr")
                    nc.tensor.transpose(tvT_ps[:KRET, :], tv, ident[:S, :S])
                    tvT = spool.tile([KRET, S], F32, name="tvT", tag="tvT")
                    nc.vector.tensor_copy(tvT, tvT_ps[:KRET, :])

                    # chained gathers (per stream): index table, then em_V rows
                    gsel = small.tile([KRET, S], U32, tag="gsel")
                    for s in range(S):
                        nc.gpsimd.indirect_dma_start(
                            out=gsel[:, s:s + 1], out_offset=None, in_=d_gtab,
                            in_offset=IndirectOffsetOnAxis(
                                ap=tcT[:, s:s + 1], axis=0))
                    gseli = small.tile([KRET, S], I32, tag="gseli")
                    nc.vector.tensor_copy(gseli, gsel)
                    vtop4 = spool.tile([KRET, S * DE], F32, name="vtop4",
                                       tag="vtop4")
                    for s in range(S):
                        nc.gpsimd.indirect_dma_start(
                            out=vtop4[:, s * DE:(s + 1) * DE], out_offset=None,
                            in_=d_V,
                            in_offset=IndirectOffsetOnAxis(
                                ap=gseli[:, s:s + 1], axis=0))

                if stage == 2:
                    for s in range(S):
                        nc.sync.dma_start(
                            d_out[s:s + 1, :].rearrange(
                                "one (p r) -> p one r", p=KRET),
                            vtop4[:, s * DE:s * DE + KRET])

                if stage >= 3:
                    # ---- phase A (batched): attention + softmax ----
                    attn4 = small.tile([KRET, S], F32, tag="attn4")
                    for s in range(S):
                        prodA = scr.tile([KRET, 1], F32, tag="prodA")
                        nc.vector.scalar_tensor_tensor(
                            out=prodA.broadcast_to([KRET, DE]),
                            in0=vtop4[:, s * DE:(s + 1) * DE],
                            scalar=float(DE ** -0.5), in1=qc_rep[s],
                            op0=OP.mult, op1=OP.mult,
                            accum_out=attn4[:, s:s + 1])
                    nc.vector.tensor_add(attn4, attn4, tvT)
                    aT_ps = pp.tile([128, KRET], F32, space="PSUM", tag="tr")
                    nc.tensor.transpose(aT_ps[:S, :], attn4, ident[:KRET, :KRET])
                    aT = small.tile([S, KRET], F32, tag="aT")
                    nc.vector.tensor_copy(aT, aT_ps[:S, :])
                    mx4 = small.tile([S, 1], F32, tag="mx4")
                    nc.vector.reduce_max(mx4, aT, axis=mybir.AxisListType.X)
                    nc.vector.tensor_scalar(aT, aT, mx4, None, op0=OP.subtract)
                    ew = small.tile([S, KRET], F32, tag="ew")
                    sume4 = small.tile([S, 1], F32, tag="sume4")
                    nc.scalar.activation(ew, aT, AF.Exp, accum_out=sume4)
                    rcp4 = small.tile([S, 1], F32, tag="rcp4")
                    nc.vector.reciprocal(rcp4, sume4)
                    nc.vector.tensor_scalar(ew, ew, rcp4, None, op0=OP.mult)
                    wT_ps = pp.tile([128, S], F32, space="PSUM", tag="tr")
                    nc.tensor.transpose(wT_ps[:KRET, :], ew, ident[:S, :S])
                    wT = small.tile([KRET, S], F32, tag="wT")
                    nc.vector.tensor_copy(wT, wT_ps[:KRET, :])
                    ps_oe = pacc.tile([128, S], F32, space="PSUM", tag="acc")
                    for s in range(S):
                        nc.tensor.matmul(ps_oe[:, s:s + 1],
                                         lhsT=vtop4[:, s * DE:(s + 1) * DE],
                                         rhs=wT[:, s:s + 1], start=True, stop=True)
                    h04 = spool.tile([128, S], F32, name="h04", tag="h04")
                    nc.vector.tensor_copy(h04, ps_oe)

                    # ---- phase B (batched): layernorm + FFN + out proj ----
                    ps_s1 = pq.tile([S, 1], F32, space="PSUM", tag="row")
                    nc.tensor.matmul(ps_s1, lhsT=h04, rhs=ones_col,
                                     start=True, stop=True)
                    mean4 = small.tile([S, 1], F32, tag="mean4")
                    nc.vector.tensor_scalar(mean4, ps_s1, 1.0 / DE, None,
                                            op0=OP.mult)
                    mr_ps = pp.tile([128, S], F32, space="PSUM", tag="tr")
                    nc.tensor.transpose(mr_ps[:1, :], mean4, ident[:S, :S])
                    mrow = small.tile([1, S], F32, tag="mrow")
                    nc.vector.tensor_copy(mrow, mr_ps[:1, :])
                    mb_ps = pp.tile([128, S], F32, space="PSUM", tag="tr")
                    nc.tensor.matmul(mb_ps, lhsT=ones_row, rhs=mrow,
                                     start=True, stop=True)
                    c4 = small.tile([128, S], F32, tag="c4")
                    nc.vector.tensor_sub(c4, h04, mb_ps)
                    ps_vv = pq.tile([S, S], F32, space="PSUM", tag="row")
                    nc.tensor.matmul(ps_vv, lhsT=c4, rhs=c4, start=True, stop=True)
                    vd = small.tile([S, S], F32, tag="vd")
                    nc.vector.tensor_mul(vd, ps_vv, ident[:S, :S])
                    var4 = small.tile([S, 1], F32, tag="var4")
                    nc.vector.reduce_sum(var4, vd, axis=mybir.AxisListType.X)
                    nc.vector.tensor_scalar(var4, var4, 1.0 / DE, None, op0=OP.mult)
                    lnv = small.tile([S, 1], F32, tag="lnv")
                    nc.scalar.activation(lnv, var4, AF.Ln, bias=eps5[:S, :])
                    nc.vector.tensor_scalar(lnv, lnv, -0.5, None, op0=OP.mult)
                    rstd4 = small.tile([S, 1], F32, tag="rstd4")
                    nc.scalar.activation(rstd4, lnv, AF.Exp)
                    rr_ps = pp.tile([128, S], F32, space="PSUM", tag="tr")
                    nc.tensor.transpose(rr_ps[:1, :], rstd4, ident[:S, :S])
                    rrow = small.tile([1, S], F32, tag="rrow")
                    nc.vector.tensor_copy(rrow, rr_ps[:1, :])
                    rb_ps = pp.tile([128, S], F32, space="PSUM", tag="tr")
                    nc.tensor.matmul(rb_ps, lhsT=ones_row, rhs=rrow,
                                     start=True, stop=True)
                    hln4 = small.tile([128, S], F32, tag="hln4")
                    nc.vector.tensor_mul(hln4, c4, rb_ps)
                    nc.vector.tensor_mul(hln4, hln4, lng_c.to_broadcast([128, S]))
                    nc.vector.tensor_add(hln4, hln4, lnb_c.to_broadcast([128, S]))

                    ps_h1 = pacc.tile([128, 4 * S], F32, space="PSUM", tag="acc")
                    for k in range(4):
                        nc.tensor.matmul(ps_h1[:, k * S:(k + 1) * S],
                                         lhsT=w1[:, k * 128:(k + 1) * 128],
                                         rhs=hln4, start=True, stop=True)
                    t14 = small.tile([128, 4 * S], F32, tag="t14")
                    for k in range(4):
                        nc.vector.tensor_add(t14[:, k * S:(k + 1) * S],
                                             ps_h1[:, k * S:(k + 1) * S],
                                             b1_c[:, k:k + 1].to_broadcast([128, S]))
                    g14 = small.tile([128, 4 * S], F32, tag="g14")
                    nc.scalar.activation(g14, t14, act_fn)

                    ps_h2 = pacc.tile([128, S], F32, space="PSUM", tag="acc")
                    for k in range(4):
                        nc.tensor.matmul(ps_h2, lhsT=w2[:, k * DE:(k + 1) * DE],
                                         rhs=g14[:, k * S:(k + 1) * S],
                                         start=(k == 0), stop=(k == 3))
                    r4 = small.tile([128, S], F32, tag="r4")
                    nc.vector.tensor_add(r4, ps_h2, b2_c.to_broadcast([128, S]))
                    nc.vector.tensor_add(r4, r4, h04)
                    y4 = small.tile([S, D], F32, tag="y4")
                    for k in range(2):
                        ps_y = pq.tile([S, 512], F32, space="PSUM", tag="row")
                        nc.tensor.matmul(ps_y, lhsT=r4,
                                         rhs=wo[:, k * 512:(k + 1) * 512],
                                         start=True, stop=True)
                        nc.vector.tensor_add(y4[:, k * 512:(k + 1) * 512], ps_y,
                                             bo4[:, k * 512:(k + 1) * 512])
                    nc.sync.dma_start(d_out, y4)

    nc.compile()
    return nc


_NC_CACHE = {}


def _get_nc(M=32768, debug=False, stage=99):
    key = (M, debug, stage)
    if key not in _NC_CACHE:
        _NC_CACHE[key] = build_nc(M=M, debug=debug, stage=stage)
    return _NC_CACHE[key]


def make_in_maps(inputs, M=32768, ncores=NCORES):
    """Split full inputs into per-core input maps."""
    JPB = min(4096, M) // 128
    shared = {
        "cst_ident": np.eye(128, dtype=np.float32),
        "cst_iota_jpb": (np.arange(128, dtype=np.uint32) * JPB)[:, None],
        "cst_iota_s": (np.arange(S, dtype=np.uint32) * 1024)[:, None],
    }
    for name in ["Wq_em_w", "Wq_em_b", "Wq_cross_w", "Wq_cross_b", "Wo_w",
                 "Wo_b", "ln_g", "ln_b", "ffn1_w", "ffn1_b", "ffn2_w", "ffn2_b"]:
        shared[name] = np.ascontiguousarray(np.asarray(inputs[name], np.float32))
    in_maps = []
    for c in range(ncores):
        sl = slice(c * S, (c + 1) * S)
        m = dict(shared)
        m["x"] = np.ascontiguousarray(np.asarray(inputs["x"][sl], np.float32))
        m["y_wm"] = np.ascontiguousarray(np.asarray(inputs["y_wm"][sl], np.float32))
        m["em_K"] = np.ascontiguousarray(
            np.asarray(inputs["em_K"][sl], np.float32).reshape(S * M, DE))
        m["em_V"] = np.ascontiguousarray(
            np.asarray(inputs["em_V"][sl], np.float32).reshape(S * M, DE))
        m["em_S"] = np.ascontiguousarray(np.asarray(inputs["em_S"][sl], np.float32))
        in_maps.append(m)
    return in_maps


def kernel(**inputs):
    from concourse.bass_utils import run_bass_kernel_spmd

    nc = _get_nc()
    in_maps = make_in_maps(inputs)
    res = run_bass_kernel_spmd(nc, in_maps, list(range(NCORES))).results
    return np.concatenate([res[c]["out"] for c in range(NCORES)], axis=0)



# revision 18
# speedup vs baseline: 1.3292x; 1.1122x over previous
"""Trainium2 Bass kernel for nn_EpisodicMemory (scatter_memory).

Sharding: pure batch data-parallelism. 8 cores, 32 streams -> 4 streams/core.
Each core runs the full per-stream pipeline:
  q projections (PE) -> masked cosine scores over M=32768 slots (DVE
  tensor_tensor_reduce, em_K consumed in natural [slot, d] layout, em_S mask
  folded in as the reduce init scalar) -> per-partition top-8 (DVE Max8) ->
  batched fold to top-32 -> chained indirect DMA gathers (index table, em_V
  rows) -> cross-attention + softmax + FFN epilogue (PE/ACT, tiny).

`stage` (debug): 1 = scoring only (dump scores), 2 = + selection/gather
(dump V_top), 99 = full.
"""

import os
import sys

import numpy as np

sys.path.insert(0, "/opt/trn_rl_repo")

import concourse.bass as bass  # noqa: F401
import concourse.tile as tile
from concourse import bacc, mybir
from concourse.bass import IndirectOffsetOnAxis
from concourse.masks import make_identity

F32 = mybir.dt.float32
I32 = mybir.dt.int32
U32 = mybir.dt.uint32
OP = mybir.AluOpType
AF = mybir.ActivationFunctionType

NCORES = 8
BS, D, DE, KRET = 32, 1024, 128, 32
S = BS // NCORES  # streams per core = 4
NEG = -3.0e30  # stand-in for -inf (safe for exp/compare, no NaNs)


def register_dot_prefix():
    """Custom DVE op: out = running prefix-sum of Src0*Src1 along the free
    stream. With a stride-0 innermost out AP, the surviving write per page
    is the prefix total at that page's end -> segmented dot products in one
    instruction per chunk (vs one scalar_tensor_tensor + accum-read per
    128-slot column)."""
    from concourse.dve_ops import (
        CUSTOM_DVE_SPECS,
        OPS,
        _CUSTOM_DVE_ROW_BASE,
        _SUB_OPCODE_FOR_NAME,
        DveOp,
    )
    from concourse.dve_spec import AluOp, Spec, Src0, Src1, lower, scan
    from concourse.dve_uop import DveOpSpec

    name = "DOT_PREFIX_ANT"
    if name in _SUB_OPCODE_FOR_NAME:
        return next(op for op in OPS if op.name == name)

    def _ref(in0, in1, s0, s1, imm2):
        p = in0.shape[0]
        a = np.asarray(in0, np.float32).reshape(p, -1)
        b = np.asarray(in1, np.float32).reshape(p, -1)
        return np.cumsum(a * b, axis=-1, dtype=np.float32).reshape(in0.shape)

    spec = Spec(body=scan(AluOp.ADD, Src0 * Src1), reference=_ref)
    row = _CUSTOM_DVE_ROW_BASE + len(OPS)
    sha = {}
    for ver in ("v3", "v4"):
        tmp = DveOpSpec(name=name, opcode=row, uops=lower(spec, ver=ver), rd1_en=True)
        sha[ver] = tmp.sha(ver)
    op = DveOp(name, spec, subdim=False, uops_sha=sha)
    OPS.append(op)
    CUSTOM_DVE_SPECS[name] = spec
    _SUB_OPCODE_FOR_NAME[name] = row
    return op


def build_nc(M=32768, debug=False, act_fn=None, stage=99, reps=1, serial_reps=False):
    """Build the per-core Bass kernel. M = slots per stream (param for sim)."""
    if act_fn is None:
        act_fn = AF.Gelu
    CH = min(4096, M)         # slots per DMA chunk (4096 slots = 2 MB)
    NCHUNK = M // CH
    JPB = CH // 128           # rows per partition per chunk (32)
    NCOL = M // 128           # score columns (256)
    NCAND = 1024              # per-stream candidates (128 partitions x 8)

    dot_op = register_dot_prefix()
    nc = bacc.Bacc("TRN2", target_bir_lowering=False, debug=debug)

    # ---- DRAM I/O (per-core shard) ----
    d_x = nc.dram_tensor("x", [S, D], F32, kind="ExternalInput").ap()
    d_y = nc.dram_tensor("y_wm", [S, D], F32, kind="ExternalInput").ap()
    d_K = nc.dram_tensor("em_K", [S * M, DE], F32, kind="ExternalInput").ap()
    d_V = nc.dram_tensor("em_V", [S * M, DE], F32, kind="ExternalInput").ap()
    d_S = nc.dram_tensor("em_S", [S, M], F32, kind="ExternalInput").ap()
    d_wqe = nc.dram_tensor("Wq_em_w", [2 * D, DE], F32, kind="ExternalInput").ap()
    d_bqe = nc.dram_tensor("Wq_em_b", [DE], F32, kind="ExternalInput").ap()
    d_wqc = nc.dram_tensor("Wq_cross_w", [D, DE], F32, kind="ExternalInput").ap()
    d_bqc = nc.dram_tensor("Wq_cross_b", [DE], F32, kind="ExternalInput").ap()
    d_wo = nc.dram_tensor("Wo_w", [DE, D], F32, kind="ExternalInput").ap()
    d_bo = nc.dram_tensor("Wo_b", [D], F32, kind="ExternalInput").ap()
    d_lng = nc.dram_tensor("ln_g", [DE], F32, kind="ExternalInput").ap()
    d_lnb = nc.dram_tensor("ln_b", [DE], F32, kind="ExternalInput").ap()
    d_w1 = nc.dram_tensor("ffn1_w", [DE, 4 * DE], F32, kind="ExternalInput").ap()
    d_b1 = nc.dram_tensor("ffn1_b", [4 * DE], F32, kind="ExternalInput").ap()
    d_w2 = nc.dram_tensor("ffn2_w", [4 * DE, DE], F32, kind="ExternalInput").ap()
    d_b2 = nc.dram_tensor("ffn2_b", [DE], F32, kind="ExternalInput").ap()
    d_out = nc.dram_tensor("out", [S, D], F32, kind="ExternalOutput").ap()
    d_ident = nc.dram_tensor("cst_ident", [128, 128], F32, kind="ExternalInput").ap()
    d_iotaj = nc.dram_tensor("cst_iota_jpb", [128, 1], U32, kind="ExternalInput").ap()
    d_iotas = nc.dram_tensor("cst_iota_s", [S, 1], U32, kind="ExternalInput").ap()
    # index table for the chained gather (slot row ids as uint32)
    d_gtab = nc.dram_tensor("gtab", [S * NCAND, 1], U32).ap()

    with tile.TileContext(nc) as tc:
        with (
            tc.tile_pool(name="kpool", bufs=7) as kpool,
            tc.tile_pool(name="wpool", bufs=1) as wpool,
            tc.tile_pool(name="spool", bufs=1) as spool,
            tc.tile_pool(name="scr", bufs=2) as scr,
            tc.tile_pool(name="small", bufs=4) as small,
            tc.tile_pool(name="pp", bufs=3, space="PSUM") as pp,
            tc.tile_pool(name="pacc", bufs=2, space="PSUM") as pacc,
            tc.tile_pool(name="pq", bufs=2, space="PSUM") as pq,
        ):
            # ---- constants / weights in SBUF ----
            ident = wpool.tile([128, 128], F32, name="ident")
            nc.sync.dma_start(ident, d_ident)
            ones_row = wpool.tile([1, 128], F32, name="ones_row")
            nc.vector.memset(ones_row, 1.0)
            ones_col = wpool.tile([128, 1], F32, name="ones_col")
            nc.vector.memset(ones_col, 1.0)
            iota32 = wpool.tile([128, 1], U32, name="iota32")  # p * JPB
            nc.sync.dma_start(iota32, d_iotaj)
            iotaS = wpool.tile([S, 1], U32, name="iotaS")  # s * NCAND
            nc.sync.dma_start(iotaS, d_iotas)
            eps12 = wpool.tile([128, 1], F32, name="eps12")
            nc.vector.memset(eps12, 1e-12)
            eps5 = wpool.tile([128, 1], F32, name="eps5")
            nc.vector.memset(eps5, 1e-5)

            # Wq_em rows 2048 -> [128, 16*128]; Wq_cross rows 1024 -> [128, 8*128]
            # Weight loads ride the gpsimd ring: their (expensive) descriptor
            # writes then don't block the sync ring's small early DMAs.
            wqe = wpool.tile([128, 16 * DE], F32, name="wqe")
            for j in range(16):
                nc.gpsimd.dma_start(wqe[:, j * DE:(j + 1) * DE],
                                    d_wqe[j * 128:(j + 1) * 128, :])
            wqc = wpool.tile([128, 8 * DE], F32, name="wqc")
            for j in range(8):
                nc.gpsimd.dma_start(wqc[:, j * DE:(j + 1) * DE],
                                    d_wqc[j * 128:(j + 1) * 128, :])
            w1 = wpool.tile([128, 512], F32, name="w1")
            nc.gpsimd.dma_start(w1, d_w1)
            w2 = wpool.tile([128, 4 * DE], F32, name="w2")
            nc.gpsimd.dma_start(w2, d_w2.rearrange("(k p) e -> p k e", p=128))
            wo = wpool.tile([128, D], F32, name="wo")
            nc.gpsimd.dma_start(wo, d_wo)
            bqe_r = wpool.tile([S, DE], F32, name="bqe_r")
            for _s in range(S):
                nc.sync.dma_start(bqe_r[_s:_s + 1, :], d_bqe[None, :])
            bqc_r = wpool.tile([S, DE], F32, name="bqc_r")
            for _s in range(S):
                nc.sync.dma_start(bqc_r[_s:_s + 1, :], d_bqc[None, :])
            # esel block s: [S,128] with row s all-ones; matmul(lhsT=esel_s,
            # rhs=X[S,:]) replicates X's row s across all 128 partitions
            esel = wpool.tile([S, S * 128], F32, name="esel")
            nc.vector.memset(esel, 0.0)
            for _s in range(S):
                nc.sync.dma_start(esel[_s:_s + 1, _s * 128:(_s + 1) * 128],
                                  ones_row)
            lng_c = wpool.tile([128, 1], F32, name="lng_c")
            nc.sync.dma_start(lng_c, d_lng[:, None])
            lnb_c = wpool.tile([128, 1], F32, name="lnb_c")
            nc.sync.dma_start(lnb_c, d_lnb[:, None])
            b1_c = wpool.tile([128, 4], F32, name="b1_c")
            nc.sync.dma_start(b1_c, d_b1.rearrange("(k p) -> p k", p=128))
            b2_c = wpool.tile([128, 1], F32, name="b2_c")
            nc.sync.dma_start(b2_c, d_b2[:, None])
            bo4 = wpool.tile([S, D], F32, name="bo4")
            for _s in range(S):
                nc.sync.dma_start(bo4[_s:_s + 1, :], d_bo[None, :])

            def bcast_col(val11, n=128):
                """[1,1] sbuf -> [n,1] sbuf via PE outer product."""
                ps = pp.tile([128, 1], F32, space="PSUM", tag="tr")
                nc.tensor.matmul(ps[:n, :], lhsT=ones_row[:, :n], rhs=val11,
                                 start=True, stop=True)
                sb = small.tile([n, 1], F32, tag="bc_sb")
                nc.vector.tensor_copy(sb, ps[:n, :])
                return sb

            def transpose(src, pdim, fdim):
                """[pdim, fdim] -> psum [fdim, pdim]; returns psum AP."""
                ps = pp.tile([128, 128], F32, space="PSUM", tag="tr")
                nc.tensor.transpose(ps[:fdim, :pdim], src, ident[:pdim, :pdim])
                return ps[:fdim, :pdim]

            def rsqrt11(val11, eps_ap, tag):
                """[1,1] -> 1/sqrt(val + eps) via exp(-0.5 * ln(val + eps))."""
                t = small.tile([1, 1], F32, tag=tag + "_ln")
                nc.scalar.activation(t, val11, AF.Ln, bias=eps_ap)
                t2 = small.tile([1, 1], F32, tag=tag + "_sc")
                nc.vector.tensor_scalar(t2, t, -0.5, None, op0=OP.mult)
                r = small.tile([1, 1], F32, tag=tag + "_ex")
                nc.scalar.activation(r, t2, AF.Exp)
                return r

            for rep_ in range(reps):
                if serial_reps and rep_ > 0:
                    fence = scr.tile([S, D], F32, tag="fence")
                    nc.sync.dma_start(fence, d_out)
                    fs = small.tile([S, 1], F32, tag="fs")
                    nc.vector.reduce_max(fs, fence, axis=mybir.AxisListType.X)
                # ---- prefetch: first kpool-bufs K chunks, issued on the
                # scalar engine's HW ring as its first instructions so the
                # DMA engines are saturated from t=0 (the sync ring is
                # blocked behind the query phase's semaphores) ----
                PREF = 7
                kt_pre = []
                if stage >= 1:
                    sc_pairs = [(s, c) for s in range(S) for c in range(NCHUNK)]
                    for (s, c) in sc_pairs[:PREF]:
                        kt = kpool.tile([128, CH], F32, tag="ktile")
                        base = s * M + c * CH
                        nc.scalar.dma_start(
                            kt, d_K[base:base + CH, :].rearrange(
                                "(p j) d -> p j d", p=128))
                        kt_pre.append(kt)

                # ---- phase 0: batched queries (all S streams at once) ----
                # qT[s, de] = sum_dd xcat[s, dd]*W[dd, de] via 16 accumulating
                # matmuls with the cheap operand (xT block, 4 cols) as weights.
                q_rep, qc_rep = [], []
                xn8 = scr.tile([S, 2 * D], F32, tag="xn8")
                nc.sync.dma_start(xn8[:, :D], d_x)
                nc.sync.dma_start(xn8[:, D:], d_y)
                xTs = []
                for j in range(16):
                    ps_t = pp.tile([128, S], F32, space="PSUM", tag="tr")
                    nc.tensor.transpose(ps_t, xn8[:, j * 128:(j + 1) * 128],
                                        ident[:S, :S])
                    xT = wpool.tile([128, S], F32, name=f"xTb{j}")
                    nc.vector.tensor_copy(xT, ps_t)
                    xTs.append(xT)

                ps_qT = pacc.tile([S, DE], F32, space="PSUM", tag="acc")
                for j in range(16):
                    nc.tensor.matmul(ps_qT, lhsT=xTs[j],
                                     rhs=wqe[:, j * DE:(j + 1) * DE],
                                     start=(j == 0), stop=(j == 15))
                qT = spool.tile([S, DE], F32, name="qT", tag="qT")
                nc.vector.tensor_add(qT, ps_qT, bqe_r)
                # unit-normalize rows of qT
                sqsc = small.tile([S, 1], F32, tag="sqsc")
                nrm = small.tile([S, 1], F32, tag="nrm")
                nc.vector.scalar_tensor_tensor(
                    out=sqsc.broadcast_to([S, DE]), in0=qT, scalar=0.0, in1=qT,
                    op0=OP.bypass, op1=OP.mult, accum_out=nrm)
                lnq = small.tile([S, 1], F32, tag="lnq")
                nc.scalar.activation(lnq, nrm, AF.Ln, bias=eps12[:S, :])
                nc.vector.tensor_scalar(lnq, lnq, -0.5, None, op0=OP.mult)
                rstq = small.tile([S, 1], F32, tag="rstq")
                nc.scalar.activation(rstq, lnq, AF.Exp)
                nc.vector.tensor_scalar(qT, qT, rstq, None, op0=OP.mult)

                # q_cross = x @ Wq_cross + b (reuses xT blocks 0..7)
                ps_qcT = pacc.tile([S, DE], F32, space="PSUM", tag="acc")
                for j in range(8):
                    nc.tensor.matmul(ps_qcT, lhsT=xTs[j],
                                     rhs=wqc[:, j * DE:(j + 1) * DE],
                                     start=(j == 0), stop=(j == 7))
                qcT = spool.tile([S, DE], F32, name="qcT", tag="qcT")
                nc.vector.tensor_add(qcT, ps_qcT, bqc_r)

                # replicate each stream's q / q_cross across partitions
                for s in range(S):
                    ps_qr = pp.tile([128, 128], F32, space="PSUM", tag="tr")
                    nc.tensor.matmul(ps_qr, lhsT=esel[:, s * 128:(s + 1) * 128],
                                     rhs=qT, start=True, stop=True)
                    qr = spool.tile([128, 128], F32, name=f"q_rep{s}", tag=f"q_rep{s}")
                    nc.vector.tensor_copy(qr, ps_qr)
                    q_rep.append(qr)
                    ps_qcr = pp.tile([128, 128], F32, space="PSUM", tag="tr")
                    nc.tensor.matmul(ps_qcr[:KRET, :],
                                     lhsT=esel[:, s * 128:s * 128 + KRET],
                                     rhs=qcT, start=True, stop=True)
                    qcr = spool.tile([KRET, 128], F32, name=f"qc_rep{s}", tag=f"qc_rep{s}")
                    nc.vector.tensor_copy(qcr, ps_qcr[:KRET, :])
                    qc_rep.append(qcr)

                if stage == 0:
                    for s in range(S):
                        nc.sync.dma_start(
                            d_out[s:s + 1, :].rearrange("one (p r) -> p one r", p=128),
                            q_rep[s][:, :8])

                # ---- masks (em_S <= 0 -> NEG), in score layout ----
                KVAR = os.environ.get("KVAR", "")
                masks = []
                for s in range(S if stage >= 1 else 0):
                    msk = spool.tile([128, NCOL], F32, name=f"mask{s}", tag=f"mask{s}")
                    if "nomaskdma" in KVAR:
                        nc.vector.memset(msk, 0.0)
                    else:
                        msrc = scr.tile([128, NCOL], F32, tag="msrc")
                        nc.sync.dma_start(
                            msrc, d_S[s].rearrange("(c p j) -> p c j", p=128, j=JPB))
                        nc.vector.tensor_scalar(msk, msrc, 0.0, NEG,
                                                op0=OP.is_le, op1=OP.mult)
                    masks.append(msk)

                # ---- scoring: chunked DMA + segmented-dot scan (custom DVE) ----
                # One DVE instruction per chunk: prefix-sum of K*q over the
                # whole [128, JPB*DE] stream; a stride-0 innermost out AP keeps
                # only the prefix at each page end. Adjacent-difference then
                # yields the per-slot dot products.
                PJ = JPB + 1  # prefix columns per chunk (col 0 stays 0)
                scores = [spool.tile([128, NCOL], F32, name=f"scores{s}", tag=f"scores{s}")
                          for s in range(S)]
                pcols = []
                for s in range(S if stage >= 1 else 0):
                    pcol = spool.tile([128, NCHUNK * PJ], F32, name=f"pcol{s}",
                                      tag=f"pcol{s}")
                    nc.vector.memset(pcol, 0.0)
                    pcols.append(pcol)
                if stage >= 2:
                    cand = spool.tile([S, NCAND], F32, name="cand", tag="cand")
                for s in range(S if stage >= 1 else 0):
                    for c in range(NCHUNK):
                        sc_idx = s * NCHUNK + c
                        if sc_idx < len(kt_pre):
                            kt = kt_pre[sc_idx]
                        else:
                            kt = kpool.tile([128, CH], F32, tag="ktile")
                            base = s * M + c * CH
                            nc.scalar.dma_start(
                                kt, d_K[base:base + CH, :].rearrange(
                                    "(p j) d -> p j d", p=128))
                        if "noscore" in KVAR:
                            continue
                        in0 = kt.rearrange("p (j d) -> p j d", d=DE)
                        in1 = q_rep[s].unsqueeze(1).broadcast_to([128, JPB, DE])
                        out3 = pcols[s][:, c * PJ + 1:c * PJ + 1 + JPB].unsqueeze(
                            2).broadcast_to([128, JPB, DE])
                        nc.vector._custom_dve(dot_op, out=out3, in0=in0, in1=in1)
                    if "noscore" in KVAR:
                        nc.vector.memset(scores[s], 0.0)
                        continue
                    # scores = prefix[j+1] - prefix[j], then mask add
                    p3 = pcols[s].rearrange("p (c j) -> p c j", j=PJ)
                    sc3 = scores[s].rearrange("p (c j) -> p c j", j=JPB)
                    nc.vector.tensor_sub(sc3, p3[:, :, 1:PJ], p3[:, :, 0:JPB])
                    nc.vector.tensor_add(scores[s], scores[s], masks[s])

                    if stage >= 2:
                        # selection stage 1 inline: per-partition top-8 for
                        # this stream overlaps the next stream's scan DMAs
                        v8 = small.tile([128, 8], F32, tag="v8")
                        nc.vector.max(out=v8, in_=scores[s])
                        c8 = small.tile([128, 8], U32, tag="c8")
                        nc.vector.max_index(out=c8, in_max=v8, in_values=scores[s])
                        # em row = s*M + (c8>>log2(JPB))*CH + p*JPB + (c8&(JPB-1))
                        jb = int(np.log2(JPB))
                        t1 = small.tile([128, 8], U32, tag="t1")
                        nc.vector.tensor_scalar(t1, c8, jb, None,
                                                op0=OP.arith_shift_right)
                        t1b = small.tile([128, 8], U32, tag="t1b")
                        nc.vector.tensor_scalar(t1b, t1, CH, s * M,
                                                op0=OP.mult, op1=OP.add)
                        t2 = small.tile([128, 8], U32, tag="t2")
                        nc.vector.tensor_scalar(t2, c8, JPB - 1, None,
                                                op0=OP.bitwise_and)
                        t3 = small.tile([128, 8], U32, tag="t3")
                        nc.vector.tensor_add(t3, t1b, t2)
                        gidx = small.tile([128, 8], U32, tag="gidx")
                        nc.vector.tensor_add(gidx, t3, iota32.to_broadcast([128, 8]))
                        # stash values + index table
                        nc.sync.dma_start(cand[s:s + 1, :], v8)
                        nc.sync.dma_start(
                            d_gtab[s * NCAND:(s + 1) * NCAND, :].rearrange(
                                "(p r) one -> p r one", p=128), gidx)

                if stage == 1:
                    for s in range(S):
                        nc.sync.dma_start(
                            d_out[s:s + 1, :].rearrange("one (p r) -> p one r", p=128),
                            scores[s][:, :8])

                if stage >= 2:
                    # ---- selection stage 2: fold 1024 -> top-32 per stream ----
                    tv = spool.tile([S, KRET], F32, name="tv", tag="tv")
                    tc_ = spool.tile([S, KRET], U32, name="tc", tag="tc")
                    for r in range(4):
                        sl = slice(8 * r, 8 * r + 8)
                        nc.vector.max(out=tv[:, sl], in_=cand)
                        nc.vector.max_index(out=tc_[:, sl], in_max=tv[:, sl],
                                            in_values=cand)
                        if r < 3:
                            nc.vector.match_replace(out=cand, in_to_replace=tv[:, sl],
                                                    in_values=cand, imm_value=NEG)
                    tcg = spool.tile([S, KRET], F32, name="tcg", tag="tcg")
                    nc.vector.tensor_add(tcg, tc_, iotaS.to_broadcast([S, KRET]))

                    # transpose tv/tcg -> columns [KRET, S]
                    tcT_ps = pp.tile([128, S], F32, space="PSUM", tag="tr")
                    nc.tensor.transpose(tcT_ps[:KRET, :], tcg, ident[:S, :S])
                    tcT = spool.tile([KRET, S], I32, name="tcT", tag="tcT")
                    nc.vector.tensor_copy(tcT, tcT_ps[:KRET, :])
                    tvT_ps = pp.tile([128, S], F32, space="PSUM", tag="tr")
                    nc.tensor.transpose(tvT_ps[:KRET, :], tv, ident[:S, :S])
                    tvT = spool.tile([KRET, S], F32, name="tvT", tag="tvT")
                    nc.vector.tensor_copy(tvT, tvT_ps[:KRET, :])

                    # chained gathers (per stream): index table, then em_V rows
                    gsel = small.tile([KRET, S], U32, tag="gsel")
                    for s in range(S):
                        nc.gpsimd.indirect_dma_start(
                            out=gsel[:, s:s + 1], out_offset=None, in_=d_gtab,
                            in_offset=IndirectOffsetOnAxis(
                                ap=tcT[:, s:s + 1], axis=0))
                    gseli = small.tile([KRET, S], I32, tag="gseli")
                    nc.vector.tensor_copy(gseli, gsel)
                    vtop4 = spool.tile([KRET, S * DE], F32, name="vtop4",
                                       tag="vtop4")
                    for s in range(S):
                        nc.gpsimd.indirect_dma_start(
                            out=vtop4[:, s * DE:(s + 1) * DE], out_offset=None,
                            in_=d_V,
                            in_offset=IndirectOffsetOnAxis(
                                ap=gseli[:, s:s + 1], axis=0))

                if stage == 2:
                    for s in range(S):
                        nc.sync.dma_start(
                            d_out[s:s + 1, :].rearrange(
                                "one (p r) -> p one r", p=KRET),
                            vtop4[:, s * DE:s * DE + KRET])

                if stage >= 3:
                    # ---- phase A (batched): attention + softmax ----
                    attn4 = small.tile([KRET, S], F32, tag="attn4")
                    for s in range(S):
                        prodA = scr.tile([KRET, 1], F32, tag="prodA")
                        nc.vector.scalar_tensor_tensor(
                            out=prodA.broadcast_to([KRET, DE]),
                            in0=vtop4[:, s * DE:(s + 1) * DE],
                            scalar=float(DE ** -0.5), in1=qc_rep[s],
                            op0=OP.mult, op1=OP.mult,
                            accum_out=attn4[:, s:s + 1])
                    nc.vector.tensor_add(attn4, attn4, tvT)
                    aT_ps = pp.tile([128, KRET], F32, space="PSUM", tag="tr")
                    nc.tensor.transpose(aT_ps[:S, :], attn4, ident[:KRET, :KRET])
                    aT = small.tile([S, KRET], F32, tag="aT")
                    nc.vector.tensor_copy(aT, aT_ps[:S, :])
                    mx4 = small.tile([S, 1], F32, tag="mx4")
                    nc.vector.reduce_max(mx4, aT, axis=mybir.AxisListType.X)
                    nc.vector.tensor_scalar(aT, aT, mx4, None, op0=OP.subtract)
                    ew = small.tile([S, KRET], F32, tag="ew")
                    sume4 = small.tile([S, 1], F32, tag="sume4")
                    nc.scalar.activation(ew, aT, AF.Exp, accum_out=sume4)
                    rcp4 = small.tile([S, 1], F32, tag="rcp4")
                    nc.vector.reciprocal(rcp4, sume4)
                    nc.vector.tensor_scalar(ew, ew, rcp4, None, op0=OP.mult)
                    wT_ps = pp.tile([128, S], F32, space="PSUM", tag="tr")
                    nc.tensor.transpose(wT_ps[:KRET, :], ew, ident[:S, :S])
                    wT = small.tile([KRET, S], F32, tag="wT")
                    nc.vector.tensor_copy(wT, wT_ps[:KRET, :])
                    ps_oe = pacc.tile([128, S], F32, space="PSUM", tag="acc")
                    for s in range(S):
                        nc.tensor.matmul(ps_oe[:, s:s + 1],
                                         lhsT=vtop4[:, s * DE:(s + 1) * DE],
                                         rhs=wT[:, s:s + 1], start=True, stop=True)
                    h04 = spool.tile([128, S], F32, name="h04", tag="h04")
                    nc.vector.tensor_copy(h04, ps_oe)

                    # ---- phase B (batched): layernorm + FFN + out proj ----
                    ps_s1 = pq.tile([S, 1], F32, space="PSUM", tag="row")
                    nc.tensor.matmul(ps_s1, lhsT=h04, rhs=ones_col,
                                     start=True, stop=True)
                    mean4 = small.tile([S, 1], F32, tag="mean4")
                    nc.vector.tensor_scalar(mean4, ps_s1, 1.0 / DE, None,
                                            op0=OP.mult)
                    mr_ps = pp.tile([128, S], F32, space="PSUM", tag="tr")
                    nc.tensor.transpose(mr_ps[:1, :], mean4, ident[:S, :S])
                    mrow = small.tile([1, S], F32, tag="mrow")
                    nc.vector.tensor_copy(mrow, mr_ps[:1, :])
                    mb_ps = pp.tile([128, S], F32, space="PSUM", tag="tr")
                    nc.tensor.matmul(mb_ps, lhsT=ones_row, rhs=mrow,
                                     start=True, stop=True)
                    c4 = small.tile([128, S], F32, tag="c4")
                    nc.vector.tensor_sub(c4, h04, mb_ps)
                    ps_vv = pq.tile([S, S], F32, space="PSUM", tag="row")
                    nc.tensor.matmul(ps_vv, lhsT=c4, rhs=c4, start=True, stop=True)
                    vd = small.tile([S, S], F32, tag="vd")
                    nc.vector.tensor_mul(vd, ps_vv, ident[:S, :S])
                    var4 = small.tile([S, 1], F32, tag="var4")
                    nc.vector.reduce_sum(var4, vd, axis=mybir.AxisListType.X)
                    nc.vector.tensor_scalar(var4, var4, 1.0 / DE, None, op0=OP.mult)
                    lnv = small.tile([S, 1], F32, tag="lnv")
                    nc.scalar.activation(lnv, var4, AF.Ln, bias=eps5[:S, :])
                    nc.vector.tensor_scalar(lnv, lnv, -0.5, None, op0=OP.mult)
                    rstd4 = small.tile([S, 1], F32, tag="rstd4")
                    nc.scalar.activation(rstd4, lnv, AF.Exp)
                    rr_ps = pp.tile([128, S], F32, space="PSUM", tag="tr")
                    nc.tensor.transpose(rr_ps[:1, :], rstd4, ident[:S, :S])
                    rrow = small.tile([1, S], F32, tag="rrow")
                    nc.vector.tensor_copy(rrow, rr_ps[:1, :])
                    rb_ps = pp.tile([128, S], F32, space="PSUM", tag="tr")
                    nc.tensor.matmul(rb_ps, lhsT=ones_row, rhs=rrow,
                                     start=True, stop=True)
                    hln4 = small.tile([128, S], F32, tag="hln4")
                    nc.vector.tensor_mul(hln4, c4, rb_ps)
                    nc.vector.tensor_mul(hln4, hln4, lng_c.to_broadcast([128, S]))
                    nc.vector.tensor_add(hln4, hln4, lnb_c.to_broadcast([128, S]))

                    ps_h1 = pacc.tile([128, 4 * S], F32, space="PSUM", tag="acc")
                    for k in range(4):
                        nc.tensor.matmul(ps_h1[:, k * S:(k + 1) * S],
                                         lhsT=w1[:, k * 128:(k + 1) * 128],
                                         rhs=hln4, start=True, stop=True)
                    t14 = small.tile([128, 4 * S], F32, tag="t14")
                    for k in range(4):
                        nc.vector.tensor_add(t14[:, k * S:(k + 1) * S],
                                             ps_h1[:, k * S:(k + 1) * S],
                                             b1_c[:, k:k + 1].to_broadcast([128, S]))
                    g14 = small.tile([128, 4 * S], F32, tag="g14")
                    nc.scalar.activation(g14, t14, act_fn)

                    ps_h2 = pacc.tile([128, S], F32, space="PSUM", tag="acc")
                    for k in range(4):
                        nc.tensor.matmul(ps_h2, lhsT=w2[:, k * DE:(k + 1) * DE],
                                         rhs=g14[:, k * S:(k + 1) * S],
                                         start=(k == 0), stop=(k == 3))
                    r4 = small.tile([128, S], F32, tag="r4")
                    nc.vector.tensor_add(r4, ps_h2, b2_c.to_broadcast([128, S]))
                    nc.vector.tensor_add(r4, r4, h04)
                    y4 = small.tile([S, D], F32, tag="y4")
                    for k in range(2):
                        ps_y = pq.tile([S, 512], F32, space="PSUM", tag="row")
                        nc.tensor.matmul(ps_y, lhsT=r4,
                                         rhs=wo[:, k * 512:(k + 1) * 512],
                                         start=True, stop=True)
                        nc.vector.tensor_add(y4[:, k * 512:(k + 1) * 512], ps_y,
                                             bo4[:, k * 512:(k + 1) * 512])
                    nc.sync.dma_start(d_out, y4)

    nc.compile()
    return nc


_NC_CACHE = {}


def _get_nc(M=32768, debug=False, stage=99):
    key = (M, debug, stage)
    if key not in _NC_CACHE:
        _NC_CACHE[key] = build_nc(M=M, debug=debug, stage=stage)
    return _NC_CACHE[key]


def make_in_maps(inputs, M=32768, ncores=NCORES):
    """Split full inputs into per-core input maps."""
    JPB = min(4096, M) // 128
    shared = {
        "cst_ident": np.eye(128, dtype=np.float32),
        "cst_iota_jpb": (np.arange(128, dtype=np.uint32) * JPB)[:, None],
        "cst_iota_s": (np.arange(S, dtype=np.uint32) * 1024)[:, None],
    }
    for name in ["Wq_em_w", "Wq_em_b", "Wq_cross_w", "Wq_cross_b", "Wo_w",
                 "Wo_b", "ln_g", "ln_b", "ffn1_w", "ffn1_b", "ffn2_w", "ffn2_b"]:
        shared[name] = np.ascontiguousarray(np.asarray(inputs[name], np.float32))
    in_maps = []
    for c in range(ncores):
        sl = slice(c * S, (c + 1) * S)
        m = dict(shared)
        m["x"] = np.ascontiguousarray(np.asarray(inputs["x"][sl], np.float32))
        m["y_wm"] = np.ascontiguousarray(np.asarray(inputs["y_wm"][sl], np.float32))
        m["em_K"] = np.ascontiguousarray(
            np.asarray(inputs["em_K"][sl], np.float32).reshape(S * M, DE))
        m["em_V"] = np.ascontiguousarray(
            np.asarray(inputs["em_V"][sl], np.float32).reshape(S * M, DE))
        m["em_S"] = np.ascontiguousarray(np.asarray(inputs["em_S"][sl], np.float32))
        in_maps.append(m)
    return in_maps


def kernel(**inputs):
    from concourse.bass_utils import run_bass_kernel_spmd

    nc = _get_nc()
    in_maps = make_in_maps(inputs)
    res = run_bass_kernel_spmd(nc, in_maps, list(range(NCORES))).results
    return np.concatenate([res[c]["out"] for c in range(NCORES)], axis=0)



# revision 24
# speedup vs baseline: 1.4174x; 1.0664x over previous
"""Trainium2 Bass kernel for nn_EpisodicMemory (scatter_memory).

Sharding: pure batch data-parallelism. 8 cores, 32 streams -> 4 streams/core.
Each core runs the full per-stream pipeline:
  q projections (PE) -> masked cosine scores over M=32768 slots (DVE
  tensor_tensor_reduce, em_K consumed in natural [slot, d] layout, em_S mask
  folded in as the reduce init scalar) -> per-partition top-8 (DVE Max8) ->
  batched fold to top-32 -> chained indirect DMA gathers (index table, em_V
  rows) -> cross-attention + softmax + FFN epilogue (PE/ACT, tiny).

`stage` (debug): 1 = scoring only (dump scores), 2 = + selection/gather
(dump V_top), 99 = full.
"""

import os
import sys

import numpy as np

sys.path.insert(0, "/opt/trn_rl_repo")

import concourse.bass as bass  # noqa: F401
import concourse.tile as tile
from concourse import bacc, mybir
from concourse.bass import IndirectOffsetOnAxis
from concourse.masks import make_identity

F32 = mybir.dt.float32
I32 = mybir.dt.int32
U32 = mybir.dt.uint32
OP = mybir.AluOpType
AF = mybir.ActivationFunctionType

NCORES = 8
BS, D, DE, KRET = 32, 1024, 128, 32
S = BS // NCORES  # streams per core = 4
NEG = -3.0e30  # stand-in for -inf (safe for exp/compare, no NaNs)


def register_dot_prefix():
    """Custom DVE op: out = running prefix-sum of Src0*Src1 along the free
    stream. With a stride-0 innermost out AP, the surviving write per page
    is the prefix total at that page's end -> segmented dot products in one
    instruction per chunk (vs one scalar_tensor_tensor + accum-read per
    128-slot column)."""
    from concourse.dve_ops import (
        CUSTOM_DVE_SPECS,
        OPS,
        _CUSTOM_DVE_ROW_BASE,
        _SUB_OPCODE_FOR_NAME,
        DveOp,
    )
    from concourse.dve_spec import AluOp, Spec, Src0, Src1, lower, scan
    from concourse.dve_uop import DveOpSpec

    name = "DOT_PREFIX_ANT"
    if name in _SUB_OPCODE_FOR_NAME:
        return (next(op for op in OPS if op.name == name),
                next(op for op in OPS if op.name == "MASK_ADD_ANT"))

    def _ref(in0, in1, s0, s1, imm2):
        p = in0.shape[0]
        a = np.asarray(in0, np.float32).reshape(p, -1)
        b = np.asarray(in1, np.float32).reshape(p, -1)
        return np.cumsum(a * b, axis=-1, dtype=np.float32).reshape(in0.shape)

    def _register(name, spec):
        row = _CUSTOM_DVE_ROW_BASE + len(OPS)
        sha = {}
        for ver in ("v3", "v4"):
            tmp = DveOpSpec(name=name, opcode=row, uops=lower(spec, ver=ver),
                            rd1_en=True)
            sha[ver] = tmp.sha(ver)
        op = DveOp(name, spec, subdim=False, uops_sha=sha)
        OPS.append(op)
        CUSTOM_DVE_SPECS[name] = spec
        _SUB_OPCODE_FOR_NAME[name] = row
        return op

    dot = _register(name, Spec(body=scan(AluOp.ADD, Src0 * Src1), reference=_ref))

    # masked add: out = (in0 <= 0)*s0 + in1  (fuses mask build + mask apply)
    def _ref_maskadd(in0, in1, s0, s1, imm2):
        p = in0.shape[0]
        a = np.asarray(in0, np.float32).reshape(p, -1)
        b = np.asarray(in1, np.float32).reshape(p, -1)
        return ((a <= 0.0) * np.float32(s0) + b).astype(np.float32).reshape(in0.shape)

    from concourse.dve_spec import C0, Zero

    maskadd = _register(
        "MASK_ADD_ANT",
        Spec(body=(Src0 <= Zero) * C0 + Src1, reference=_ref_maskadd))
    return dot, maskadd


def build_nc(M=32768, debug=False, act_fn=None, stage=99, reps=1, serial_reps=False):
    """Build the per-core Bass kernel. M = slots per stream (param for sim)."""
    if act_fn is None:
        act_fn = AF.Gelu
    CH = min(4096, M)         # slots per DMA chunk (4096 slots = 2 MB)
    NCHUNK = M // CH
    JPB = CH // 128           # rows per partition per chunk (32)
    NCOL = M // 128           # score columns (256)
    NCAND = 1024              # per-stream candidates (128 partitions x 8)

    dot_op, maskadd_op = register_dot_prefix()
    nc = bacc.Bacc("TRN2", target_bir_lowering=False, debug=debug)

    # ---- DRAM I/O (per-core shard) ----
    d_x = nc.dram_tensor("x", [S, D], F32, kind="ExternalInput").ap()
    d_y = nc.dram_tensor("y_wm", [S, D], F32, kind="ExternalInput").ap()
    d_K = nc.dram_tensor("em_K", [S * M, DE], F32, kind="ExternalInput").ap()
    d_V = nc.dram_tensor("em_V", [S * M, DE], F32, kind="ExternalInput").ap()
    d_S = nc.dram_tensor("em_S", [S, M], F32, kind="ExternalInput").ap()
    d_wqe = nc.dram_tensor("Wq_em_w", [2 * D, DE], F32, kind="ExternalInput").ap()
    d_bqe = nc.dram_tensor("Wq_em_b", [DE], F32, kind="ExternalInput").ap()
    d_wqc = nc.dram_tensor("Wq_cross_w", [D, DE], F32, kind="ExternalInput").ap()
    d_bqc = nc.dram_tensor("Wq_cross_b", [DE], F32, kind="ExternalInput").ap()
    d_wo = nc.dram_tensor("Wo_w", [DE, D], F32, kind="ExternalInput").ap()
    d_bo = nc.dram_tensor("Wo_b", [D], F32, kind="ExternalInput").ap()
    d_lng = nc.dram_tensor("ln_g", [DE], F32, kind="ExternalInput").ap()
    d_lnb = nc.dram_tensor("ln_b", [DE], F32, kind="ExternalInput").ap()
    d_w1 = nc.dram_tensor("ffn1_w", [DE, 4 * DE], F32, kind="ExternalInput").ap()
    d_b1 = nc.dram_tensor("ffn1_b", [4 * DE], F32, kind="ExternalInput").ap()
    d_w2 = nc.dram_tensor("ffn2_w", [4 * DE, DE], F32, kind="ExternalInput").ap()
    d_b2 = nc.dram_tensor("ffn2_b", [DE], F32, kind="ExternalInput").ap()
    d_out = nc.dram_tensor("out", [S, D], F32, kind="ExternalOutput").ap()
    d_ident = nc.dram_tensor("cst_ident", [128, 128], F32, kind="ExternalInput").ap()
    d_iotaj = nc.dram_tensor("cst_iota_jpb", [128, 1], U32, kind="ExternalInput").ap()
    d_iotas = nc.dram_tensor("cst_iota_s", [S, 1], U32, kind="ExternalInput").ap()
    # index table for the chained gather (slot row ids as uint32)
    d_gtab = nc.dram_tensor("gtab", [S * NCAND, 1], U32).ap()

    with tile.TileContext(nc) as tc:
        with (
            tc.tile_pool(name="kpool", bufs=7) as kpool,
            tc.tile_pool(name="wpool", bufs=1) as wpool,
            tc.tile_pool(name="spool", bufs=1) as spool,
            tc.tile_pool(name="scr", bufs=2) as scr,
            tc.tile_pool(name="small", bufs=4) as small,
            tc.tile_pool(name="pp", bufs=3, space="PSUM") as pp,
            tc.tile_pool(name="pacc", bufs=2, space="PSUM") as pacc,
            tc.tile_pool(name="pq", bufs=2, space="PSUM") as pq,
        ):
            # ---- constants / weights in SBUF ----
            ident = wpool.tile([128, 128], F32, name="ident")
            nc.sync.dma_start(ident, d_ident)
            ones_row = wpool.tile([1, 128], F32, name="ones_row")
            nc.vector.memset(ones_row, 1.0)
            ones_col = wpool.tile([128, 1], F32, name="ones_col")
            nc.vector.memset(ones_col, 1.0)
            iota32 = wpool.tile([128, 1], U32, name="iota32")  # p * JPB
            nc.sync.dma_start(iota32, d_iotaj)
            iotaS = wpool.tile([S, 1], U32, name="iotaS")  # s * NCAND
            nc.sync.dma_start(iotaS, d_iotas)
            eps12 = wpool.tile([128, 1], F32, name="eps12")
            nc.vector.memset(eps12, 1e-12)
            eps5 = wpool.tile([128, 1], F32, name="eps5")
            nc.vector.memset(eps5, 1e-5)

            # Wq_em rows 2048 -> [128, 16*128]; Wq_cross rows 1024 -> [128, 8*128]
            # Weight loads ride the gpsimd ring: their (expensive) descriptor
            # writes then don't block the sync ring's small early DMAs.
            wqe = wpool.tile([128, 16 * DE], F32, name="wqe")
            for j in range(16):
                nc.gpsimd.dma_start(wqe[:, j * DE:(j + 1) * DE],
                                    d_wqe[j * 128:(j + 1) * 128, :])
            wqc = wpool.tile([128, 8 * DE], F32, name="wqc")
            for j in range(8):
                nc.gpsimd.dma_start(wqc[:, j * DE:(j + 1) * DE],
                                    d_wqc[j * 128:(j + 1) * 128, :])
            w1 = wpool.tile([128, 512], F32, name="w1")
            nc.gpsimd.dma_start(w1, d_w1)
            w2 = wpool.tile([128, 4 * DE], F32, name="w2")
            nc.gpsimd.dma_start(w2, d_w2.rearrange("(k p) e -> p k e", p=128))
            wo = wpool.tile([128, D], F32, name="wo")
            nc.gpsimd.dma_start(wo, d_wo)
            bqe_r = wpool.tile([S, DE], F32, name="bqe_r")
            for _s in range(S):
                nc.sync.dma_start(bqe_r[_s:_s + 1, :], d_bqe[None, :])
            bqc_r = wpool.tile([S, DE], F32, name="bqc_r")
            for _s in range(S):
                nc.sync.dma_start(bqc_r[_s:_s + 1, :], d_bqc[None, :])
            # esel block s: [S,128] with row s all-ones; matmul(lhsT=esel_s,
            # rhs=X[S,:]) replicates X's row s across all 128 partitions
            esel = wpool.tile([S, S * 128], F32, name="esel")
            nc.vector.memset(esel, 0.0)
            for _s in range(S):
                nc.sync.dma_start(esel[_s:_s + 1, _s * 128:(_s + 1) * 128],
                                  ones_row)
            lng_c = wpool.tile([128, 1], F32, name="lng_c")
            nc.sync.dma_start(lng_c, d_lng[:, None])
            lnb_c = wpool.tile([128, 1], F32, name="lnb_c")
            nc.sync.dma_start(lnb_c, d_lnb[:, None])
            b1_c = wpool.tile([128, 4], F32, name="b1_c")
            nc.sync.dma_start(b1_c, d_b1.rearrange("(k p) -> p k", p=128))
            b2_c = wpool.tile([128, 1], F32, name="b2_c")
            nc.sync.dma_start(b2_c, d_b2[:, None])
            bo4 = wpool.tile([S, D], F32, name="bo4")
            for _s in range(S):
                nc.sync.dma_start(bo4[_s:_s + 1, :], d_bo[None, :])

            def bcast_col(val11, n=128):
                """[1,1] sbuf -> [n,1] sbuf via PE outer product."""
                ps = pp.tile([128, 1], F32, space="PSUM", tag="tr")
                nc.tensor.matmul(ps[:n, :], lhsT=ones_row[:, :n], rhs=val11,
                                 start=True, stop=True)
                sb = small.tile([n, 1], F32, tag="bc_sb")
                nc.vector.tensor_copy(sb, ps[:n, :])
                return sb

            def transpose(src, pdim, fdim):
                """[pdim, fdim] -> psum [fdim, pdim]; returns psum AP."""
                ps = pp.tile([128, 128], F32, space="PSUM", tag="tr")
                nc.tensor.transpose(ps[:fdim, :pdim], src, ident[:pdim, :pdim])
                return ps[:fdim, :pdim]

            def rsqrt11(val11, eps_ap, tag):
                """[1,1] -> 1/sqrt(val + eps) via exp(-0.5 * ln(val + eps))."""
                t = small.tile([1, 1], F32, tag=tag + "_ln")
                nc.scalar.activation(t, val11, AF.Ln, bias=eps_ap)
                t2 = small.tile([1, 1], F32, tag=tag + "_sc")
                nc.vector.tensor_scalar(t2, t, -0.5, None, op0=OP.mult)
                r = small.tile([1, 1], F32, tag=tag + "_ex")
                nc.scalar.activation(r, t2, AF.Exp)
                return r

            for rep_ in range(reps):
                if serial_reps and rep_ > 0:
                    fence = scr.tile([S, D], F32, tag="fence")
                    nc.sync.dma_start(fence, d_out)
                    fs = small.tile([S, 1], F32, tag="fs")
                    nc.vector.reduce_max(fs, fence, axis=mybir.AxisListType.X)
                # ---- prefetch: first kpool-bufs K chunks, issued on the
                # scalar engine's HW ring as its first instructions so the
                # DMA engines are saturated from t=0 (the sync ring is
                # blocked behind the query phase's semaphores) ----
                PREF = 7
                kt_pre = []
                if stage >= 1:
                    sc_pairs = [(s, c) for s in range(S) for c in range(NCHUNK)]
                    for (s, c) in sc_pairs[:PREF]:
                        kt = kpool.tile([128, CH], F32, tag="ktile")
                        base = s * M + c * CH
                        nc.scalar.dma_start(
                            kt, d_K[base:base + CH, :].rearrange(
                                "(p j) d -> p j d", p=128))
                        kt_pre.append(kt)

                # ---- phase 0: batched queries (all S streams at once) ----
                # qT[s, de] = sum_dd xcat[s, dd]*W[dd, de] via 16 accumulating
                # matmuls with the cheap operand (xT block, 4 cols) as weights.
                q_rep, qc_rep = [], []
                xn8 = scr.tile([S, 2 * D], F32, tag="xn8")
                nc.sync.dma_start(xn8[:, :D], d_x)
                nc.sync.dma_start(xn8[:, D:], d_y)
                xTs = []
                for j in range(16):
                    ps_t = pp.tile([128, S], F32, space="PSUM", tag="tr")
                    nc.tensor.transpose(ps_t, xn8[:, j * 128:(j + 1) * 128],
                                        ident[:S, :S])
                    xT = wpool.tile([128, S], F32, name=f"xTb{j}")
                    nc.vector.tensor_copy(xT, ps_t)
                    xTs.append(xT)

                ps_qT = pacc.tile([S, DE], F32, space="PSUM", tag="acc")
                for j in range(16):
                    nc.tensor.matmul(ps_qT, lhsT=xTs[j],
                                     rhs=wqe[:, j * DE:(j + 1) * DE],
                                     start=(j == 0), stop=(j == 15))
                qT = spool.tile([S, DE], F32, name="qT", tag="qT")
                nc.vector.tensor_add(qT, ps_qT, bqe_r)
                # unit-normalize rows of qT
                sqsc = small.tile([S, 1], F32, tag="sqsc")
                nrm = small.tile([S, 1], F32, tag="nrm")
                nc.vector.scalar_tensor_tensor(
                    out=sqsc.broadcast_to([S, DE]), in0=qT, scalar=0.0, in1=qT,
                    op0=OP.bypass, op1=OP.mult, accum_out=nrm)
                lnq = small.tile([S, 1], F32, tag="lnq")
                nc.scalar.activation(lnq, nrm, AF.Ln, bias=eps12[:S, :])
                nc.vector.tensor_scalar(lnq, lnq, -0.5, None, op0=OP.mult)
                rstq = small.tile([S, 1], F32, tag="rstq")
                nc.scalar.activation(rstq, lnq, AF.Exp)
                nc.vector.tensor_scalar(qT, qT, rstq, None, op0=OP.mult)

                # q_cross = x @ Wq_cross + b (reuses xT blocks 0..7)
                ps_qcT = pacc.tile([S, DE], F32, space="PSUM", tag="acc")
                for j in range(8):
                    nc.tensor.matmul(ps_qcT, lhsT=xTs[j],
                                     rhs=wqc[:, j * DE:(j + 1) * DE],
                                     start=(j == 0), stop=(j == 7))
                qcT = spool.tile([S, DE], F32, name="qcT", tag="qcT")
                nc.vector.tensor_add(qcT, ps_qcT, bqc_r)

                # replicate each stream's q / q_cross across partitions
                for s in range(S):
                    ps_qr = pp.tile([128, 128], F32, space="PSUM", tag="tr")
                    nc.tensor.matmul(ps_qr, lhsT=esel[:, s * 128:(s + 1) * 128],
                                     rhs=qT, start=True, stop=True)
                    qr = spool.tile([128, 128], F32, name=f"q_rep{s}", tag=f"q_rep{s}")
                    nc.vector.tensor_copy(qr, ps_qr)
                    q_rep.append(qr)
                    ps_qcr = pp.tile([128, 128], F32, space="PSUM", tag="tr")
                    nc.tensor.matmul(ps_qcr[:KRET, :],
                                     lhsT=esel[:, s * 128:s * 128 + KRET],
                                     rhs=qcT, start=True, stop=True)
                    qcr = spool.tile([KRET, 128], F32, name=f"qc_rep{s}", tag=f"qc_rep{s}")
                    nc.vector.tensor_copy(qcr, ps_qcr[:KRET, :])
                    qc_rep.append(qcr)

                if stage == 0:
                    for s in range(S):
                        nc.sync.dma_start(
                            d_out[s:s + 1, :].rearrange("one (p r) -> p one r", p=128),
                            q_rep[s][:, :8])

                # ---- em_S loads in score layout; mask fused into scoring via
                # MASK_ADD_ANT so no DVE work lands before the first scan ----
                KVAR = os.environ.get("KVAR", "")
                msrcs = []
                for s in range(S if stage >= 1 else 0):
                    msrc = spool.tile([128, NCOL], F32, name=f"msrc{s}",
                                      tag=f"msrc{s}")
                    nc.sync.dma_start(
                        msrc, d_S[s].rearrange("(c p j) -> p c j", p=128, j=JPB))
                    msrcs.append(msrc)

                # ---- scoring: chunked DMA + segmented-dot scan (custom DVE) ----
                # One DVE instruction per chunk: prefix-sum of K*q over the
                # whole [128, JPB*DE] stream; a stride-0 innermost out AP keeps
                # only the prefix at each page end. Adjacent-difference then
                # yields the per-slot dot products.
                PJ = JPB + 1  # prefix columns per chunk (col 0 stays 0)
                scores = [spool.tile([128, NCOL], F32, name=f"scores{s}", tag=f"scores{s}")
                          for s in range(S)]
                pcols = []
                for s in range(S if stage >= 1 else 0):
                    pcol = spool.tile([128, NCHUNK * PJ], F32, name=f"pcol{s}",
                                      tag=f"pcol{s}")
                    nc.vector.memset(pcol, 0.0)
                    pcols.append(pcol)
                if stage >= 2:
                    cand = spool.tile([S, NCAND], F32, name="cand", tag="cand")
                for s in range(S if stage >= 1 else 0):
                    for c in range(NCHUNK):
                        sc_idx = s * NCHUNK + c
                        if sc_idx < len(kt_pre):
                            kt = kt_pre[sc_idx]
                        else:
                            kt = kpool.tile([128, CH], F32, tag="ktile")
                            base = s * M + c * CH
                            nc.scalar.dma_start(
                                kt, d_K[base:base + CH, :].rearrange(
                                    "(p j) d -> p j d", p=128))
                        if "noscore" in KVAR:
                            continue
                        in0 = kt.rearrange("p (j d) -> p j d", d=DE)
                        in1 = q_rep[s].unsqueeze(1).broadcast_to([128, JPB, DE])
                        out3 = pcols[s][:, c * PJ + 1:c * PJ + 1 + JPB].unsqueeze(
                            2).broadcast_to([128, JPB, DE])
                        nc.vector._custom_dve(dot_op, out=out3, in0=in0, in1=in1)
                    if "noscore" in KVAR:
                        nc.vector.memset(scores[s], 0.0)
                        continue
                    # scores = prefix[j+1] - prefix[j], then fused mask add
                    p3 = pcols[s].rearrange("p (c j) -> p c j", j=PJ)
                    sc3 = scores[s].rearrange("p (c j) -> p c j", j=JPB)
                    nc.vector.tensor_sub(sc3, p3[:, :, 1:PJ], p3[:, :, 0:JPB])
                    nc.vector._custom_dve(maskadd_op, out=scores[s],
                                          in0=msrcs[s], in1=scores[s], s0=NEG)

                    if stage >= 2:
                        # selection stage 1 inline: per-partition top-8 for
                        # this stream overlaps the next stream's scan DMAs
                        v8 = small.tile([128, 8], F32, tag="v8")
                        nc.vector.max(out=v8, in_=scores[s])
                        c8 = small.tile([128, 8], U32, tag="c8")
                        nc.vector.max_index(out=c8, in_max=v8, in_values=scores[s])
                        # em row = s*M + (c8>>log2(JPB))*CH + p*JPB + (c8&(JPB-1))
                        jb = int(np.log2(JPB))
                        t1 = small.tile([128, 8], U32, tag="t1")
                        nc.vector.tensor_scalar(t1, c8, jb, None,
                                                op0=OP.arith_shift_right)
                        t1b = small.tile([128, 8], U32, tag="t1b")
                        nc.vector.tensor_scalar(t1b, t1, CH, s * M,
                                                op0=OP.mult, op1=OP.add)
                        t2 = small.tile([128, 8], U32, tag="t2")
                        nc.vector.tensor_scalar(t2, c8, JPB - 1, None,
                                                op0=OP.bitwise_and)
                        t3 = small.tile([128, 8], U32, tag="t3")
                        nc.vector.tensor_add(t3, t1b, t2)
                        gidx = small.tile([128, 8], U32, tag="gidx")
                        nc.vector.tensor_add(gidx, t3, iota32.to_broadcast([128, 8]))
                        # stash values + index table
                        nc.sync.dma_start(cand[s:s + 1, :], v8)
                        nc.sync.dma_start(
                            d_gtab[s * NCAND:(s + 1) * NCAND, :].rearrange(
                                "(p r) one -> p r one", p=128), gidx)

                if stage == 1:
                    for s in range(S):
                        nc.sync.dma_start(
                            d_out[s:s + 1, :].rearrange("one (p r) -> p one r", p=128),
                            scores[s][:, :8])

                if stage >= 2:
                    # ---- selection stage 2: fold 1024 -> top-32 per stream ----
                    tv = spool.tile([S, KRET], F32, name="tv", tag="tv")
                    tc_ = spool.tile([S, KRET], U32, name="tc", tag="tc")
                    for r in range(4):
                        sl = slice(8 * r, 8 * r + 8)
                        nc.vector.max(out=tv[:, sl], in_=cand)
                        nc.vector.max_index(out=tc_[:, sl], in_max=tv[:, sl],
                                            in_values=cand)
                        if r < 3:
                            nc.vector.match_replace(out=cand, in_to_replace=tv[:, sl],
                                                    in_values=cand, imm_value=NEG)
                    tcg = spool.tile([S, KRET], F32, name="tcg", tag="tcg")
                    nc.vector.tensor_add(tcg, tc_, iotaS.to_broadcast([S, KRET]))

                    # transpose tv/tcg -> columns [KRET, S]
                    tcT_ps = pp.tile([128, S], F32, space="PSUM", tag="tr")
                    nc.tensor.transpose(tcT_ps[:KRET, :], tcg, ident[:S, :S])
                    tcT = spool.tile([KRET, S], I32, name="tcT", tag="tcT")
                    nc.vector.tensor_copy(tcT, tcT_ps[:KRET, :])
                    tvT_ps = pp.tile([128, S], F32, space="PSUM", tag="tr")
                    nc.tensor.transpose(tvT_ps[:KRET, :], tv, ident[:S, :S])
                    tvT = spool.tile([KRET, S], F32, name="tvT", tag="tvT")
                    nc.vector.tensor_copy(tvT, tvT_ps[:KRET, :])

                    # chained gathers (per stream): index table, then em_V rows
                    gsel = small.tile([KRET, S], U32, tag="gsel")
                    for s in range(S):
                        nc.gpsimd.indirect_dma_start(
                            out=gsel[:, s:s + 1], out_offset=None, in_=d_gtab,
                            in_offset=IndirectOffsetOnAxis(
                                ap=tcT[:, s:s + 1], axis=0))
                    gseli = small.tile([KRET, S], I32, tag="gseli")
                    nc.vector.tensor_copy(gseli, gsel)
                    vtop4 = spool.tile([KRET, S * DE], F32, name="vtop4",
                                       tag="vtop4")
                    for s in range(S):
                        nc.gpsimd.indirect_dma_start(
                            out=vtop4[:, s * DE:(s + 1) * DE], out_offset=None,
                            in_=d_V,
                            in_offset=IndirectOffsetOnAxis(
                                ap=gseli[:, s:s + 1], axis=0))

                if stage == 2:
                    for s in range(S):
                        nc.sync.dma_start(
                            d_out[s:s + 1, :].rearrange(
                                "one (p r) -> p one r", p=KRET),
                            vtop4[:, s * DE:s * DE + KRET])

                if stage >= 3:
                    # ---- phase A (batched): attention + softmax ----
                    attn4 = small.tile([KRET, S], F32, tag="attn4")
                    for s in range(S):
                        prodA = scr.tile([KRET, 1], F32, tag="prodA")
                        nc.vector.scalar_tensor_tensor(
                            out=prodA.broadcast_to([KRET, DE]),
                            in0=vtop4[:, s * DE:(s + 1) * DE],
                            scalar=float(DE ** -0.5), in1=qc_rep[s],
                            op0=OP.mult, op1=OP.mult,
                            accum_out=attn4[:, s:s + 1])
                    nc.vector.tensor_add(attn4, attn4, tvT)
                    aT_ps = pp.tile([128, KRET], F32, space="PSUM", tag="tr")
                    nc.tensor.transpose(aT_ps[:S, :], attn4, ident[:KRET, :KRET])
                    aT = small.tile([S, KRET], F32, tag="aT")
                    nc.vector.tensor_copy(aT, aT_ps[:S, :])
                    mx4 = small.tile([S, 1], F32, tag="mx4")
                    nc.vector.reduce_max(mx4, aT, axis=mybir.AxisListType.X)
                    nc.vector.tensor_scalar(aT, aT, mx4, None, op0=OP.subtract)
                    ew = small.tile([S, KRET], F32, tag="ew")
                    sume4 = small.tile([S, 1], F32, tag="sume4")
                    nc.scalar.activation(ew, aT, AF.Exp, accum_out=sume4)
                    rcp4 = small.tile([S, 1], F32, tag="rcp4")
                    nc.vector.reciprocal(rcp4, sume4)
                    nc.vector.tensor_scalar(ew, ew, rcp4, None, op0=OP.mult)
                    wT_ps = pp.tile([128, S], F32, space="PSUM", tag="tr")
                    nc.tensor.transpose(wT_ps[:KRET, :], ew, ident[:S, :S])
                    wT = small.tile([KRET, S], F32, tag="wT")
                    nc.vector.tensor_copy(wT, wT_ps[:KRET, :])
                    ps_oe = pacc.tile([128, S], F32, space="PSUM", tag="acc")
                    for s in range(S):
                        nc.tensor.matmul(ps_oe[:, s:s + 1],
                                         lhsT=vtop4[:, s * DE:(s + 1) * DE],
                                         rhs=wT[:, s:s + 1], start=True, stop=True)
                    h04 = spool.tile([128, S], F32, name="h04", tag="h04")
                    nc.vector.tensor_copy(h04, ps_oe)

                    # ---- phase B (batched): layernorm + FFN + out proj ----
                    ps_s1 = pq.tile([S, 1], F32, space="PSUM", tag="row")
                    nc.tensor.matmul(ps_s1, lhsT=h04, rhs=ones_col,
                                     start=True, stop=True)
                    mean4 = small.tile([S, 1], F32, tag="mean4")
                    nc.vector.tensor_scalar(mean4, ps_s1, 1.0 / DE, None,
                                            op0=OP.mult)
                    mr_ps = pp.tile([128, S], F32, space="PSUM", tag="tr")
                    nc.tensor.transpose(mr_ps[:1, :], mean4, ident[:S, :S])
                    mrow = small.tile([1, S], F32, tag="mrow")
                    nc.vector.tensor_copy(mrow, mr_ps[:1, :])
                    mb_ps = pp.tile([128, S], F32, space="PSUM", tag="tr")
                    nc.tensor.matmul(mb_ps, lhsT=ones_row, rhs=mrow,
                                     start=True, stop=True)
                    c4 = small.tile([128, S], F32, tag="c4")
                    nc.vector.tensor_sub(c4, h04, mb_ps)
                    ps_vv = pq.tile([S, S], F32, space="PSUM", tag="row")
                    nc.tensor.matmul(ps_vv, lhsT=c4, rhs=c4, start=True, stop=True)
                    vd = small.tile([S, S], F32, tag="vd")
                    nc.vector.tensor_mul(vd, ps_vv, ident[:S, :S])
                    var4 = small.tile([S, 1], F32, tag="var4")
                    nc.vector.reduce_sum(var4, vd, axis=mybir.AxisListType.X)
                    nc.vector.tensor_scalar(var4, var4, 1.0 / DE, None, op0=OP.mult)
                    lnv = small.tile([S, 1], F32, tag="lnv")
                    nc.scalar.activation(lnv, var4, AF.Ln, bias=eps5[:S, :])
                    nc.vector.tensor_scalar(lnv, lnv, -0.5, None, op0=OP.mult)
                    rstd4 = small.tile([S, 1], F32, tag="rstd4")
                    nc.scalar.activation(rstd4, lnv, AF.Exp)
                    rr_ps = pp.tile([128, S], F32, space="PSUM", tag="tr")
                    nc.tensor.transpose(rr_ps[:1, :], rstd4, ident[:S, :S])
                    rrow = small.tile([1, S], F32, tag="rrow")
                    nc.vector.tensor_copy(rrow, rr_ps[:1, :])
                    rb_ps = pp.tile([128, S], F32, space="PSUM", tag="tr")
                    nc.tensor.matmul(rb_ps, lhsT=ones_row, rhs=rrow,
                                     start=True, stop=True)
                    hln4 = small.tile([128, S], F32, tag="hln4")
                    nc.vector.tensor_mul(hln4, c4, rb_ps)
                    nc.vector.tensor_mul(hln4, hln4, lng_c.to_broadcast([128, S]))
                    nc.vector.tensor_add(hln4, hln4, lnb_c.to_broadcast([128, S]))

                    ps_h1 = pacc.tile([128, 4 * S], F32, space="PSUM", tag="acc")
                    for k in range(4):
                        nc.tensor.matmul(ps_h1[:, k * S:(k + 1) * S],
                                         lhsT=w1[:, k * 128:(k + 1) * 128],
                                         rhs=hln4, start=True, stop=True)
                    t14 = small.tile([128, 4 * S], F32, tag="t14")
                    for k in range(4):
                        nc.vector.tensor_add(t14[:, k * S:(k + 1) * S],
                                             ps_h1[:, k * S:(k + 1) * S],
                                             b1_c[:, k:k + 1].to_broadcast([128, S]))
                    g14 = small.tile([128, 4 * S], F32, tag="g14")
                    nc.scalar.activation(g14, t14, act_fn)

                    ps_h2 = pacc.tile([128, S], F32, space="PSUM", tag="acc")
                    for k in range(4):
                        nc.tensor.matmul(ps_h2, lhsT=w2[:, k * DE:(k + 1) * DE],
                                         rhs=g14[:, k * S:(k + 1) * S],
                                         start=(k == 0), stop=(k == 3))
                    r4 = small.tile([128, S], F32, tag="r4")
                    nc.vector.tensor_add(r4, ps_h2, b2_c.to_broadcast([128, S]))
                    nc.vector.tensor_add(r4, r4, h04)
                    y4 = small.tile([S, D], F32, tag="y4")
                    for k in range(2):
                        ps_y = pq.tile([S, 512], F32, space="PSUM", tag="row")
                        nc.tensor.matmul(ps_y, lhsT=r4,
                                         rhs=wo[:, k * 512:(k + 1) * 512],
                                         start=True, stop=True)
                        nc.vector.tensor_add(y4[:, k * 512:(k + 1) * 512], ps_y,
                                             bo4[:, k * 512:(k + 1) * 512])
                    nc.sync.dma_start(d_out, y4)

    nc.compile()
    return nc


_NC_CACHE = {}


def _get_nc(M=32768, debug=False, stage=99):
    key = (M, debug, stage)
    if key not in _NC_CACHE:
        _NC_CACHE[key] = build_nc(M=M, debug=debug, stage=stage)
    return _NC_CACHE[key]


def make_in_maps(inputs, M=32768, ncores=NCORES):
    """Split full inputs into per-core input maps."""
    JPB = min(4096, M) // 128
    shared = {
        "cst_ident": np.eye(128, dtype=np.float32),
        "cst_iota_jpb": (np.arange(128, dtype=np.uint32) * JPB)[:, None],
        "cst_iota_s": (np.arange(S, dtype=np.uint32) * 1024)[:, None],
    }
    for name in ["Wq_em_w", "Wq_em_b", "Wq_cross_w", "Wq_cross_b", "Wo_w",
                 "Wo_b", "ln_g", "ln_b", "ffn1_w", "ffn1_b", "ffn2_w", "ffn2_b"]:
        shared[name] = np.ascontiguousarray(np.asarray(inputs[name], np.float32))
    in_maps = []
    for c in range(ncores):
        sl = slice(c * S, (c + 1) * S)
        m = dict(shared)
        m["x"] = np.ascontiguousarray(np.asarray(inputs["x"][sl], np.float32))
        m["y_wm"] = np.ascontiguousarray(np.asarray(inputs["y_wm"][sl], np.float32))
        m["em_K"] = np.ascontiguousarray(
            np.asarray(inputs["em_K"][sl], np.float32).reshape(S * M, DE))
        m["em_V"] = np.ascontiguousarray(
            np.asarray(inputs["em_V"][sl], np.float32).reshape(S * M, DE))
        m["em_S"] = np.ascontiguousarray(np.asarray(inputs["em_S"][sl], np.float32))
        in_maps.append(m)
    return in_maps


def kernel(**inputs):
    from concourse.bass_utils import run_bass_kernel_spmd

    nc = _get_nc()
    in_maps = make_in_maps(inputs)
    res = run_bass_kernel_spmd(nc, in_maps, list(range(NCORES))).results
    return np.concatenate([res[c]["out"] for c in range(NCORES)], axis=0)

